# revision 12
# baseline (speedup 1.0000x reference)
"""Trainium2 Bass kernel for nn_EquivSetConv (hypergraph message passing).

Reference computation:
    Xve = (X @ W1 + b1)[vertex]
    Xe  = segment_sum(Xve, edges, M)
    Xev = Xe[edges]
    H   = concat([X[vertex], Xev], -1) @ W2 + b2
    Xv  = segment_sum(H, vertex, N)
    out = ((1-a)*Xv + a*X0) @ W3 + b3

Algebraic restructure (A[v,e] = #incidence pairs (v,e)):
    Se  = A^T @ X                          (segmented sum of raw X rows per edge)
    Xe  = Se @ W1 + edeg x b1
    T   = A @ Se                           (segmented sum of Se rows per vertex)
    Xv  = deg . (X @ W2a) + T @ (W1 @ W2b) + deg x b2 + wdeg x (b1 @ W2b)
    out = ((1-a)Xv + a X0) @ W3 + b3

So the 800k-row dense matmul disappears; the kernel is two sparse
gather+segmented-sum stages plus small dense matmuls.

Sharding over 8 cores: stage A partitioned by edge range (each core owns
M/8 edges and all pairs incident to them -> computes its Se slice fully,
no cross-core reduction), one AllGather of Se, stage B partitioned by
vertex range (each core owns N/8 vertices -> computes its output rows
end to end). The only collective is the 1.6MB/rank AllGather.

Sparse stages on device: host sorts pairs by destination segment and
packs them into 128-pair chunks that are pure in a 128-wide segment
window.  For each chunk: dma_gather 128 source rows (pair p -> SBUF
partition p), build a one-hot [pair, segment-slot] matrix on DVE
(iota == seg), and accumulate with one PE matmul into the window's PSUM
tile.  Windows flush to SBUF/DRAM when complete.
"""

import numpy as np

P = 128
D = 128


# ---------------------------------------------------------------------------
# host-side preprocessing
# ---------------------------------------------------------------------------

def _pack_stream(seg_local, gidx, n_windows, chunk_counts):
    """Pack pairs (sorted by window) into window-pure 128-slot chunks.

    seg_local: [n] int, segment id LOCAL to the stream's window grid
               (seg_local // 128 = window, seg_local % 128 = slot)
    gidx:      [n] int, gather index of each pair
    chunk_counts: [n_windows] int, chunks allocated per window (shared
               across all cores so the program structure is identical).

    Returns (idx16, segf) flat arrays of length sum(chunk_counts)*128,
    pad slots have idx 0 / seg -1.
    """
    total_chunks = int(np.sum(chunk_counts))
    tot = total_chunks * P
    idx16 = np.zeros(tot, dtype=np.int16)
    segf = np.full(tot, -1.0, dtype=np.float32)
    if len(seg_local) == 0:
        return idx16, segf

    order = np.argsort(seg_local, kind="stable")
    seg_s = seg_local[order]
    gidx_s = gidx[order]
    win = seg_s // P

    # position of each pair: chunk_base[win]*128 + rank-within-window
    chunk_base = np.concatenate([[0], np.cumsum(chunk_counts)[:-1]])
    win_start = np.searchsorted(win, np.arange(n_windows), side="left")
    rank = np.arange(len(win)) - win_start[win]
    pos = chunk_base[win] * P + rank
    idx16[pos] = gidx_s.astype(np.int16)
    segf[pos] = (seg_s % P).astype(np.float32)
    return idx16, segf


def _wrap_idx(idx16, G):
    """Reshape a flat per-stream idx array into the dma_gather SBUF layout.

    Within each batch of G*128 indices, index i lives at
    [partition i%16, column i//16]; batches are side by side.
    Output [128, total_chunks*8] int16 (rows 0..15 replicated to 128).
    """
    nb = len(idx16) // (G * P)
    blocks = [idx16[b * G * P:(b + 1) * G * P].reshape(G * 8, 16).T for b in range(nb)]
    arr16 = np.hstack(blocks)  # [16, total_chunks*8]
    return np.tile(arr16, (8, 1)).astype(np.int16)


def _seg_tile(segf):
    """[total_chunks*128] -> [128, total_chunks]: pair (chunk c, part p)."""
    n_chunks = len(segf) // P
    return np.ascontiguousarray(segf.reshape(n_chunks, P).T)


def _chunk_counts(windows_per_core, n_windows, G, min_one=True):
    """windows_per_core: list over cores of [n_windows] pair counts.
    Returns per-window chunk counts (max over cores), padded so the
    total is a multiple of G."""
    counts = np.zeros(n_windows, dtype=np.int64)
    for wc in windows_per_core:
        counts = np.maximum(counts, (wc + P - 1) // P)
    if min_one:
        counts = np.maximum(counts, 1)
    tot = int(counts.sum())
    rem = (-tot) % G
    counts[-1] += rem
    return counts


def preprocess(X, vertex, edges, X0, W1, b1, W2, b2, W3, b3,
               M=25000, ncores=8, G=32, lo_split=32768):
    """Build per-core input maps + compile-time metadata."""
    X = np.asarray(X, dtype=np.float32)
    X0 = np.asarray(X0, dtype=np.float32)
    vertex = np.asarray(vertex).astype(np.int64)
    edges = np.asarray(edges).astype(np.int64)
    W1 = np.asarray(W1, dtype=np.float32)
    b1 = np.asarray(b1, dtype=np.float32)
    W2 = np.asarray(W2, dtype=np.float32)
    b2 = np.asarray(b2, dtype=np.float32)
    W3 = np.asarray(W3, dtype=np.float32)
    b3 = np.asarray(b3, dtype=np.float32)

    N, Din = X.shape
    NNZ = len(vertex)
    Dout = W3.shape[1]
    assert Din == D and Dout == D

    LO = min(lo_split, N)
    EPC = M // ncores
    VPC = N // ncores
    assert M % ncores == 0 and N % ncores == 0
    NW2 = (EPC + P - 1) // P
    NW3 = (VPC + P - 1) // P

    alpha = 0.5
    W2a = W2[:D]
    W2b = W2[D:]
    deg = np.bincount(vertex, minlength=N).astype(np.float64)
    edeg = np.bincount(edges, minlength=M).astype(np.float64)
    wdeg = np.bincount(vertex, weights=edeg[edges], minlength=N)

    Wa = ((1.0 - alpha) * W2a).astype(np.float32)
    Wt = ((1.0 - alpha) * (W1.astype(np.float64) @ W2b.astype(np.float64))).astype(np.float32)
    b1w = (W2b.astype(np.float64).T @ b1.astype(np.float64))  # b1 @ W2b
    b3_full = np.tile(b3[None, :], (P, 1)).astype(np.float32)

    core_edge = edges // EPC
    core_vert = vertex // VPC

    # ---- per-core pair lists
    s2lo_w, s2hi_w, s3_w = [], [], []     # per-core window pair-counts
    s2lo_pairs, s2hi_pairs, s3_pairs = [], [], []
    for i in range(ncores):
        sel = np.nonzero(core_edge == i)[0]
        pv = vertex[sel]
        pe = edges[sel] - i * EPC
        mlo = pv < LO
        for store_w, store_p, v, e in (
            (s2lo_w, s2lo_pairs, pv[mlo], pe[mlo]),
            (s2hi_w, s2hi_pairs, pv[~mlo] - LO, pe[~mlo]),
        ):
            store_w.append(np.bincount(e // P, minlength=NW2))
            store_p.append((e, v))
        sel = np.nonzero(core_vert == i)[0]
        pe = edges[sel]
        pvl = vertex[sel] - i * VPC
        s3_w.append(np.bincount(pvl // P, minlength=NW3))
        s3_pairs.append((pvl, pe))

    C2lo = _chunk_counts(s2lo_w, NW2, G)
    C2hi = _chunk_counts(s2hi_w, NW2, G)
    C3 = _chunk_counts(s3_w, NW3, G)
    S2LO, S2HI, S3 = int(C2lo.sum()), int(C2hi.sum()), int(C3.sum())

    iota = np.tile(np.arange(P, dtype=np.float32), G)[None, :].repeat(P, axis=0)
    iota = np.ascontiguousarray(iota)

    in_maps = []
    for i in range(ncores):
        e, v = s2lo_pairs[i]
        lo_idx, lo_seg = _pack_stream(e, v, NW2, C2lo)
        e, v = s2hi_pairs[i]
        hi_idx, hi_seg = _pack_stream(e, v, NW2, C2hi)
        pvl, pe = s3_pairs[i]
        s3_idx, s3_seg = _pack_stream(pvl, pe, NW3, C3)

        sl = slice(i * VPC, (i + 1) * VPC)
        xd_t = np.ascontiguousarray((X[sl].astype(np.float64) * deg[sl, None]).T).astype(np.float32)
        x0h = alpha * X0[sl].astype(np.float64).T \
            + (1.0 - alpha) * (np.outer(b2, deg[sl]) + np.outer(b1w, wdeg[sl]))
        x0h_t = np.ascontiguousarray(x0h).astype(np.float32)

        in_maps.append({
            "x_tab": X,
            "s2lo_idx": _wrap_idx(lo_idx, G), "s2lo_seg": _seg_tile(lo_seg),
            "s2hi_idx": _wrap_idx(hi_idx, G), "s2hi_seg": _seg_tile(hi_seg),
            "s3_idx": _wrap_idx(s3_idx, G), "s3_seg": _seg_tile(s3_seg),
            "iota": iota,
            "xd_t": xd_t,
            "x0h_t": x0h_t,
            "wa": Wa, "wt": Wt, "w3": W3, "b3f": b3_full,
        })

    meta = dict(N=N, M=M, NNZ=NNZ, ncores=ncores, G=G, LO=LO,
                EPC=EPC, VPC=VPC, NW2=NW2, NW3=NW3,
                C2lo=C2lo.tolist(), C2hi=C2hi.tolist(), C3=C3.tolist(),
                S2LO=S2LO, S2HI=S2HI, S3=S3)
    return in_maps, meta


# ---------------------------------------------------------------------------
# device program
# ---------------------------------------------------------------------------

def build_program(meta):
    import concourse.bacc as bacc
    import concourse.bass as bass  # noqa: F401
    import concourse.mybir as mybir
    import concourse.tile as tile
    from concourse._compat import get_trn_type
    from concourse import library_config
    from concourse.tile_rust import add_dep_helper

    f32 = mybir.dt.float32
    i16 = mybir.dt.int16

    ncores = meta["ncores"]
    G = meta["G"]
    N, M = meta["N"], meta["M"]
    LO = meta["LO"]
    EPC, VPC = meta["EPC"], meta["VPC"]
    NW2, NW3 = meta["NW2"], meta["NW3"]
    C2lo, C2hi, C3 = meta["C2lo"], meta["C2hi"], meta["C3"]
    S2LO, S2HI, S3 = meta["S2LO"], meta["S2HI"], meta["S3"]
    GP = G * P

    nc = bacc.Bacc(get_trn_type() or "TRN2", num_devices=ncores)

    x_tab = nc.declare_dram_parameter("x_tab", [N, D], f32, isOutput=False)
    s2lo_idx = nc.declare_dram_parameter("s2lo_idx", [P, S2LO * 8], i16, isOutput=False)
    s2lo_seg = nc.declare_dram_parameter("s2lo_seg", [P, S2LO], f32, isOutput=False)
    s2hi_idx = nc.declare_dram_parameter("s2hi_idx", [P, S2HI * 8], i16, isOutput=False)
    s2hi_seg = nc.declare_dram_parameter("s2hi_seg", [P, S2HI], f32, isOutput=False)
    s3_idx = nc.declare_dram_parameter("s3_idx", [P, S3 * 8], i16, isOutput=False)
    s3_seg = nc.declare_dram_parameter("s3_seg", [P, S3], f32, isOutput=False)
    iota_d = nc.declare_dram_parameter("iota", [P, GP], f32, isOutput=False)
    xd_d = nc.declare_dram_parameter("xd_t", [D, VPC], f32, isOutput=False)
    x0h_d = nc.declare_dram_parameter("x0h_t", [D, VPC], f32, isOutput=False)
    wa_d = nc.declare_dram_parameter("wa", [D, D], f32, isOutput=False)
    wt_d = nc.declare_dram_parameter("wt", [D, D], f32, isOutput=False)
    w3_d = nc.declare_dram_parameter("w3", [D, D], f32, isOutput=False)
    b3f_d = nc.declare_dram_parameter("b3f", [P, D], f32, isOutput=False)
    out_d = nc.declare_dram_parameter("out", [VPC, D], f32, isOutput=True)

    se_slice = nc.dram_tensor("se_slice", [EPC, D], f32)
    se_full = nc.dram_tensor("se_full", [M, D], f32, addr_space="Shared")

    with tile.TileContext(nc) as tc:
        with (
            tc.tile_pool(name="consts", bufs=1) as consts,
            tc.tile_pool(name="resident", bufs=1) as resident,
            tc.tile_pool(name="gat", bufs=2) as gat,
            tc.tile_pool(name="ohp", bufs=2) as ohp,
            tc.tile_pool(name="sep", bufs=3) as sep,
            tc.tile_pool(name="winp", bufs=3, space="PSUM") as winp,
            tc.tile_pool(name="zvp", bufs=2, space="PSUM") as zvp,
            tc.tile_pool(name="outp", bufs=2, space="PSUM") as outp,
        ):
            # ---- resident loads
            iota_t = consts.tile([P, G, P], f32)
            nc.sync.dma_start(iota_t[:], iota_d[:].rearrange("p (g q) -> p g q", q=P))
            wa_t = consts.tile([D, D], f32)
            nc.sync.dma_start(wa_t[:], wa_d[:])
            wt_t = consts.tile([D, D], f32)
            nc.sync.dma_start(wt_t[:], wt_d[:])
            w3_t = consts.tile([D, D], f32)
            nc.sync.dma_start(w3_t[:], w3_d[:])
            b3f_t = consts.tile([P, D], f32)
            nc.sync.dma_start(b3f_t[:], b3f_d[:])

            nc.gpsimd.load_library(library_config.mlp)
            npairs_reg = nc.gpsimd.to_reg(GP)

            class Stream:
                def __init__(self, name, idx_d, seg_d, n_chunks, table_ap, counts):
                    self.name = name
                    self.counts = counts
                    self.off = np.concatenate([[0], np.cumsum(counts)[:-1]]).astype(int)
                    self.table_ap = table_ap
                    self.idx_t = resident.tile([P, n_chunks * 8], i16, tag=f"idx_{name}")
                    nc.sync.dma_start(self.idx_t[:], idx_d[:])
                    self.seg_t = resident.tile([P, n_chunks], f32, tag=f"seg_{name}")
                    nc.sync.dma_start(self.seg_t[:], seg_d[:])
                    self.batches = {}
                    self.gather_insts = []

                def batch(self, b):
                    if b not in self.batches:
                        gt = gat.tile([P, G, D], f32, tag="gat")
                        inst = nc.gpsimd.dma_gather(
                            gt[:],
                            self.table_ap,
                            self.idx_t[:, b * G * 8:(b + 1) * G * 8],
                            GP,
                            npairs_reg,
                            D,
                        )
                        self.gather_insts.append(inst)
                        oh = ohp.tile([P, G, P], f32, tag="oh")
                        nc.vector.tensor_tensor(
                            out=oh[:],
                            in0=iota_t[:],
                            in1=self.seg_t[:, b * G:(b + 1) * G].broadcast_to([P, G, P]),
                            op=mybir.AluOpType.is_equal,
                        )
                        self.batches[b] = (gt, oh)
                    return self.batches[b]

            lo = Stream("s2lo", s2lo_idx, s2lo_seg, S2LO, x_tab[0:LO, :], C2lo)
            streams2 = [lo]
            if LO < N:
                hi = Stream("s2hi", s2hi_idx, s2hi_seg, S2HI, x_tab[LO:N, :], C2hi)
                streams2.append(hi)

            # ---- stage A: Se[e] = sum_{pairs with edge e} X[v]
            for w in range(NW2):
                total_k = sum(int(s.counts[w]) for s in streams2)
                psum_w = winp.tile([P, P], f32, tag="win")
                k = 0
                for s in streams2:
                    for c in range(int(s.off[w]), int(s.off[w]) + int(s.counts[w])):
                        b, cl = divmod(c, G)
                        gt, oh = s.batch(b)
                        nc.tensor.matmul(
                            psum_w[:],
                            lhsT=oh[:, cl, :],
                            rhs=gt[:, cl, :],
                            start=(k == 0),
                            stop=(k == total_k - 1),
                        )
                        k += 1
                rows = min(P, EPC - w * P)
                st = sep.tile([P, P], f32, tag="seflush")
                nc.vector.tensor_copy(out=st[:], in_=psum_w[:])
                nc.sync.dma_start(out=se_slice[w * P:w * P + rows, :], in_=st[:rows, :])

            # ---- AllGather Se
            ag = nc.gpsimd.collective_compute(
                "AllGather",
                mybir.AluOpType.bypass,
                replica_groups=[list(range(ncores))],
                ins=[se_slice[:]],
                outs=[se_full[:]],
            )

            # ---- stage B: T[v] = sum_{pairs with vertex v} Se[e] (transposed)
            s3 = Stream("s3", s3_idx, s3_seg, S3, se_full[:], C3)
            Tt = resident.tile([P, NW3 * P], f32, tag="Tt")
            for w in range(NW3):
                total_k = int(s3.counts[w])
                psum_w = winp.tile([P, P], f32, tag="win")
                for k, c in enumerate(range(int(s3.off[w]), int(s3.off[w]) + total_k)):
                    b, cl = divmod(c, G)
                    gt, oh = s3.batch(b)
                    nc.tensor.matmul(
                        psum_w[:],
                        lhsT=gt[:, cl, :],
                        rhs=oh[:, cl, :],
                        start=(k == 0),
                        stop=(k == total_k - 1),
                    )
                nc.vector.tensor_copy(out=Tt[:, w * P:(w + 1) * P], in_=psum_w[:])
            for inst in s3.gather_insts:
                add_dep_helper(inst.ins, ag.ins, reason="stage-B gathers read AllGathered Se")

            # ---- stage C: Zt = (1-a)*Xv^T + (a*X0 + affine)^T
            xd_t = resident.tile([D, VPC], f32, tag="xd")
            nc.sync.dma_start(xd_t[:], xd_d[:])
            x0h_t = resident.tile([D, VPC], f32, tag="x0h")
            nc.sync.dma_start(x0h_t[:], x0h_d[:])

            RT = 512
            for rt in range((VPC + RT - 1) // RT):
                s0 = rt * RT
                L = min(RT, VPC - s0)
                pz = zvp.tile([P, RT], f32, tag="zv")
                nc.tensor.matmul(pz[:, :L], lhsT=wa_t[:], rhs=xd_t[:, s0:s0 + L],
                                 start=True, stop=False)
                nc.tensor.matmul(pz[:, :L], lhsT=wt_t[:], rhs=Tt[:, s0:s0 + L],
                                 start=False, stop=True)
                nc.vector.tensor_add(out=x0h_t[:, s0:s0 + L], in0=pz[:, :L],
                                     in1=x0h_t[:, s0:s0 + L])

            # ---- stage D: out rows = Zt_tile^T @ W3 + b3
            for ot in range(NW3):
                s0 = ot * P
                L = min(P, VPC - s0)
                po = outp.tile([P, P], f32, tag="out")
                nc.tensor.matmul(po[:L, :], lhsT=x0h_t[:, s0:s0 + L], rhs=w3_t[:],
                                 start=True, stop=True)
                st = sep.tile([P, P], f32, tag="outflush")
                nc.vector.tensor_tensor(out=st[:L, :], in0=po[:L, :], in1=b3f_t[:L, :],
                                        op=mybir.AluOpType.add)
                nc.sync.dma_start(out=out_d[s0:s0 + L, :], in_=st[:L, :])

    return nc


# ---------------------------------------------------------------------------
# entry point
# ---------------------------------------------------------------------------

def _run(inputs, trace=False, M=25000, ncores=8, G=32, lo_split=32768):
    import sys
    if "/opt/trn_rl_repo" not in sys.path:
        sys.path.insert(0, "/opt/trn_rl_repo")
    from concourse.bass_utils import run_bass_kernel_spmd

    in_maps, meta = preprocess(**inputs, M=M, ncores=ncores, G=G, lo_split=lo_split)
    nc = build_program(meta)
    if not nc.is_finalized():
        nc.finalize()
    res = run_bass_kernel_spmd(nc, in_maps, list(range(ncores)), trace=trace)
    out = np.concatenate([np.asarray(res.results[i]["out"]) for i in range(ncores)], axis=0)
    return out, res


def kernel(**inputs):
    out, _ = _run(inputs)
    return out


# revision 13
# speedup vs baseline: 2.5882x; 2.5882x over previous
"""Trainium2 Bass kernel for nn_EquivSetConv (hypergraph message passing).

Reference computation:
    Xve = (X @ W1 + b1)[vertex]
    Xe  = segment_sum(Xve, edges, M)
    Xev = Xe[edges]
    H   = concat([X[vertex], Xev], -1) @ W2 + b2
    Xv  = segment_sum(H, vertex, N)
    out = ((1-a)*Xv + a*X0) @ W3 + b3

Algebraic restructure (A[v,e] = #incidence pairs (v,e)):
    Se  = A^T @ X                          (segmented sum of raw X rows per edge)
    Xe  = Se @ W1 + edeg x b1
    T   = A @ Se                           (segmented sum of Se rows per vertex)
    Xv  = deg . (X @ W2a) + T @ (W1 @ W2b) + deg x b2 + wdeg x (b1 @ W2b)
    out = ((1-a)Xv + a X0) @ W3 + b3

So the 800k-row dense matmul disappears; the kernel is two sparse
gather+segmented-sum stages plus small dense matmuls.

Sharding over 8 cores: stage A partitioned by edge range (each core owns
M/8 edges and all pairs incident to them -> computes its Se slice fully,
no cross-core reduction), one AllGather of Se, stage B partitioned by
vertex range (each core owns N/8 vertices -> computes its output rows
end to end). The only collective is the 1.6MB/rank AllGather.

Sparse stages on device: host sorts pairs by destination segment and
packs them into 128-pair chunks that are pure in a 128-wide segment
window.  For each chunk: dma_gather 128 source rows (pair p -> SBUF
partition p), build a one-hot [pair, segment-slot] matrix on DVE
(iota == seg), and accumulate with one PE matmul into the window's PSUM
tile.  Windows flush to SBUF/DRAM when complete.
"""

import numpy as np

P = 128
D = 128


# ---------------------------------------------------------------------------
# host-side preprocessing
# ---------------------------------------------------------------------------

def _pack_stream(seg_local, gidx, n_windows, chunk_counts):
    """Pack pairs (sorted by window) into window-pure 128-slot chunks.

    seg_local: [n] int, segment id LOCAL to the stream's window grid
               (seg_local // 128 = window, seg_local % 128 = slot)
    gidx:      [n] int, gather index of each pair
    chunk_counts: [n_windows] int, chunks allocated per window (shared
               across all cores so the program structure is identical).

    Returns (idx16, segf) flat arrays of length sum(chunk_counts)*128,
    pad slots have idx 0 / seg -1.
    """
    total_chunks = int(np.sum(chunk_counts))
    tot = total_chunks * P
    idx16 = np.zeros(tot, dtype=np.int16)
    segf = np.full(tot, -1.0, dtype=np.float32)
    if len(seg_local) == 0:
        return idx16, segf

    order = np.argsort(seg_local, kind="stable")
    seg_s = seg_local[order]
    gidx_s = gidx[order]
    win = seg_s // P

    # position of each pair: chunk_base[win]*128 + rank-within-window
    chunk_base = np.concatenate([[0], np.cumsum(chunk_counts)[:-1]])
    win_start = np.searchsorted(win, np.arange(n_windows), side="left")
    rank = np.arange(len(win)) - win_start[win]
    pos = chunk_base[win] * P + rank
    idx16[pos] = gidx_s.astype(np.int16)
    segf[pos] = (seg_s % P).astype(np.float32)
    return idx16, segf


def _wrap_idx(idx16, G):
    """Reshape a flat per-stream idx array into the dma_gather SBUF layout.

    Within each batch of G*128 indices, index i lives at
    [partition i%16, column i//16]; batches are side by side.
    Output [128, total_chunks*8] int16 (rows 0..15 replicated to 128).
    """
    nb = len(idx16) // (G * P)
    blocks = [idx16[b * G * P:(b + 1) * G * P].reshape(G * 8, 16).T for b in range(nb)]
    arr16 = np.hstack(blocks)  # [16, total_chunks*8]
    return np.tile(arr16, (8, 1)).astype(np.int16)


def _seg_tile(segf):
    """[total_chunks*128] -> [128, total_chunks]: pair (chunk c, part p)."""
    n_chunks = len(segf) // P
    return np.ascontiguousarray(segf.reshape(n_chunks, P).T)


def _chunk_counts(windows_per_core, n_windows, G, min_one=True):
    """windows_per_core: list over cores of [n_windows] pair counts.
    Returns per-window chunk counts (max over cores), padded so the
    total is a multiple of G."""
    counts = np.zeros(n_windows, dtype=np.int64)
    for wc in windows_per_core:
        counts = np.maximum(counts, (wc + P - 1) // P)
    if min_one:
        counts = np.maximum(counts, 1)
    tot = int(counts.sum())
    rem = (-tot) % G
    counts[-1] += rem
    return counts


def preprocess(X, vertex, edges, X0, W1, b1, W2, b2, W3, b3,
               M=25000, ncores=8, G=8, lo_split=32768):
    """Build per-core input maps + compile-time metadata."""
    X = np.asarray(X, dtype=np.float32)
    X0 = np.asarray(X0, dtype=np.float32)
    vertex = np.asarray(vertex).astype(np.int64)
    edges = np.asarray(edges).astype(np.int64)
    W1 = np.asarray(W1, dtype=np.float32)
    b1 = np.asarray(b1, dtype=np.float32)
    W2 = np.asarray(W2, dtype=np.float32)
    b2 = np.asarray(b2, dtype=np.float32)
    W3 = np.asarray(W3, dtype=np.float32)
    b3 = np.asarray(b3, dtype=np.float32)

    N, Din = X.shape
    NNZ = len(vertex)
    Dout = W3.shape[1]
    assert Din == D and Dout == D

    LO = min(lo_split, N)
    EPC = M // ncores
    VPC = N // ncores
    assert M % ncores == 0 and N % ncores == 0
    NW2 = (EPC + P - 1) // P
    NW3 = (VPC + P - 1) // P

    alpha = 0.5
    W2a = W2[:D]
    W2b = W2[D:]
    deg = np.bincount(vertex, minlength=N).astype(np.float64)
    edeg = np.bincount(edges, minlength=M).astype(np.float64)
    wdeg = np.bincount(vertex, weights=edeg[edges], minlength=N)

    Wa = ((1.0 - alpha) * W2a).astype(np.float32)
    Wt = ((1.0 - alpha) * (W1.astype(np.float64) @ W2b.astype(np.float64))).astype(np.float32)
    b1w = (W2b.astype(np.float64).T @ b1.astype(np.float64))  # b1 @ W2b
    b3_full = np.tile(b3[None, :], (P, 1)).astype(np.float32)

    core_edge = edges // EPC
    core_vert = vertex // VPC

    # ---- per-core pair lists
    s2lo_w, s2hi_w, s3_w = [], [], []     # per-core window pair-counts
    s2lo_pairs, s2hi_pairs, s3_pairs = [], [], []
    for i in range(ncores):
        sel = np.nonzero(core_edge == i)[0]
        pv = vertex[sel]
        pe = edges[sel] - i * EPC
        mlo = pv < LO
        for store_w, store_p, v, e in (
            (s2lo_w, s2lo_pairs, pv[mlo], pe[mlo]),
            (s2hi_w, s2hi_pairs, pv[~mlo] - LO, pe[~mlo]),
        ):
            store_w.append(np.bincount(e // P, minlength=NW2))
            store_p.append((e, v))
        sel = np.nonzero(core_vert == i)[0]
        pe = edges[sel]
        pvl = vertex[sel] - i * VPC
        s3_w.append(np.bincount(pvl // P, minlength=NW3))
        s3_pairs.append((pvl, pe))

    C2lo = _chunk_counts(s2lo_w, NW2, G)
    C2hi = _chunk_counts(s2hi_w, NW2, G)
    C3 = _chunk_counts(s3_w, NW3, G)
    S2LO, S2HI, S3 = int(C2lo.sum()), int(C2hi.sum()), int(C3.sum())

    iota = np.tile(np.arange(P, dtype=np.float32), G)[None, :].repeat(P, axis=0)
    iota = np.ascontiguousarray(iota)

    import ml_dtypes
    X_bf16 = X.astype(ml_dtypes.bfloat16)

    in_maps = []
    for i in range(ncores):
        e, v = s2lo_pairs[i]
        lo_idx, lo_seg = _pack_stream(e, v, NW2, C2lo)
        e, v = s2hi_pairs[i]
        hi_idx, hi_seg = _pack_stream(e, v, NW2, C2hi)
        pvl, pe = s3_pairs[i]
        s3_idx, s3_seg = _pack_stream(pvl, pe, NW3, C3)

        sl = slice(i * VPC, (i + 1) * VPC)
        xd_t = np.ascontiguousarray((X[sl].astype(np.float64) * deg[sl, None]).T).astype(np.float32)
        x0h = alpha * X0[sl].astype(np.float64).T \
            + (1.0 - alpha) * (np.outer(b2, deg[sl]) + np.outer(b1w, wdeg[sl]))
        x0h_t = np.ascontiguousarray(x0h).astype(np.float32)

        in_maps.append({
            "x_tab": X_bf16,
            "s2lo_idx": _wrap_idx(lo_idx, G), "s2lo_seg": _seg_tile(lo_seg),
            "s2hi_idx": _wrap_idx(hi_idx, G), "s2hi_seg": _seg_tile(hi_seg),
            "s3_idx": _wrap_idx(s3_idx, G), "s3_seg": _seg_tile(s3_seg),
            "iota": iota,
            "xd_t": xd_t,
            "x0h_t": x0h_t,
            "wa": Wa, "wt": Wt, "w3": W3, "b3f": b3_full,
        })

    meta = dict(N=N, M=M, NNZ=NNZ, ncores=ncores, G=G, LO=LO,
                EPC=EPC, VPC=VPC, NW2=NW2, NW3=NW3,
                C2lo=C2lo.tolist(), C2hi=C2hi.tolist(), C3=C3.tolist(),
                S2LO=S2LO, S2HI=S2HI, S3=S3)
    return in_maps, meta


# ---------------------------------------------------------------------------
# device program
# ---------------------------------------------------------------------------

def build_program(meta):
    import concourse.bacc as bacc
    import concourse.bass as bass  # noqa: F401
    import concourse.mybir as mybir
    import concourse.tile as tile
    from concourse._compat import get_trn_type
    from concourse import library_config
    from concourse.tile_rust import add_dep_helper

    f32 = mybir.dt.float32
    bf16 = mybir.dt.bfloat16
    i16 = mybir.dt.int16

    ncores = meta["ncores"]
    G = meta["G"]
    N, M = meta["N"], meta["M"]
    LO = meta["LO"]
    EPC, VPC = meta["EPC"], meta["VPC"]
    NW2, NW3 = meta["NW2"], meta["NW3"]
    C2lo, C2hi, C3 = meta["C2lo"], meta["C2hi"], meta["C3"]
    S2LO, S2HI, S3 = meta["S2LO"], meta["S2HI"], meta["S3"]
    GP = G * P

    nc = bacc.Bacc(get_trn_type() or "TRN2", num_devices=ncores, num_swdge_queues=4)

    x_tab = nc.declare_dram_parameter("x_tab", [N, D], bf16, isOutput=False)
    s2lo_idx = nc.declare_dram_parameter("s2lo_idx", [P, S2LO * 8], i16, isOutput=False)
    s2lo_seg = nc.declare_dram_parameter("s2lo_seg", [P, S2LO], f32, isOutput=False)
    s2hi_idx = nc.declare_dram_parameter("s2hi_idx", [P, S2HI * 8], i16, isOutput=False)
    s2hi_seg = nc.declare_dram_parameter("s2hi_seg", [P, S2HI], f32, isOutput=False)
    s3_idx = nc.declare_dram_parameter("s3_idx", [P, S3 * 8], i16, isOutput=False)
    s3_seg = nc.declare_dram_parameter("s3_seg", [P, S3], f32, isOutput=False)
    iota_d = nc.declare_dram_parameter("iota", [P, GP], f32, isOutput=False)
    xd_d = nc.declare_dram_parameter("xd_t", [D, VPC], f32, isOutput=False)
    x0h_d = nc.declare_dram_parameter("x0h_t", [D, VPC], f32, isOutput=False)
    wa_d = nc.declare_dram_parameter("wa", [D, D], f32, isOutput=False)
    wt_d = nc.declare_dram_parameter("wt", [D, D], f32, isOutput=False)
    w3_d = nc.declare_dram_parameter("w3", [D, D], f32, isOutput=False)
    b3f_d = nc.declare_dram_parameter("b3f", [P, D], f32, isOutput=False)
    out_d = nc.declare_dram_parameter("out", [VPC, D], f32, isOutput=True)

    se_slice = nc.dram_tensor("se_slice", [EPC, D], bf16)
    se_full = nc.dram_tensor("se_full", [M, D], bf16, addr_space="Shared")

    with tile.TileContext(nc) as tc:
        with (
            tc.tile_pool(name="consts", bufs=1) as consts,
            tc.tile_pool(name="resident", bufs=1) as resident,
            tc.tile_pool(name="gat", bufs=4) as gat,
            tc.tile_pool(name="ohp", bufs=4) as ohp,
            tc.tile_pool(name="sep", bufs=3) as sep,
            tc.tile_pool(name="winp", bufs=3, space="PSUM") as winp,
            tc.tile_pool(name="zvp", bufs=2, space="PSUM") as zvp,
            tc.tile_pool(name="outp", bufs=2, space="PSUM") as outp,
        ):
            # ---- resident loads
            iota_t = consts.tile([P, G, P], f32)
            nc.sync.dma_start(iota_t[:], iota_d[:].rearrange("p (g q) -> p g q", q=P))
            wa_t = consts.tile([D, D], f32)
            nc.sync.dma_start(wa_t[:], wa_d[:])
            wt_t = consts.tile([D, D], f32)
            nc.sync.dma_start(wt_t[:], wt_d[:])
            w3_t = consts.tile([D, D], f32)
            nc.sync.dma_start(w3_t[:], w3_d[:])
            b3f_t = consts.tile([P, D], f32)
            nc.sync.dma_start(b3f_t[:], b3f_d[:])

            nc.gpsimd.load_library(library_config.mlp)
            npairs_reg = nc.gpsimd.to_reg(GP)
            qctr = [0]

            class Stream:
                def __init__(self, name, idx_d, seg_d, n_chunks, table_ap, counts):
                    self.name = name
                    self.counts = counts
                    self.off = np.concatenate([[0], np.cumsum(counts)[:-1]]).astype(int)
                    self.table_ap = table_ap
                    self.idx_t = resident.tile([P, n_chunks * 8], i16, tag=f"idx_{name}")
                    nc.sync.dma_start(self.idx_t[:], idx_d[:])
                    self.seg_t = resident.tile([P, n_chunks], f32, tag=f"seg_{name}")
                    nc.sync.dma_start(self.seg_t[:], seg_d[:])
                    self.batches = {}
                    self.gather_insts = []

                def batch(self, b):
                    if b not in self.batches:
                        gt = gat.tile([P, G, D], bf16, tag="gat")
                        inst = nc.gpsimd.dma_gather(
                            gt[:],
                            self.table_ap,
                            self.idx_t[:, b * G * 8:(b + 1) * G * 8],
                            GP,
                            npairs_reg,
                            D,
                            queue_num=qctr[0] % 4,
                        )
                        qctr[0] += 1
                        self.gather_insts.append(inst)
                        oh = ohp.tile([P, G, P], bf16, tag="oh")
                        nc.vector.tensor_tensor(
                            out=oh[:],
                            in0=iota_t[:],
                            in1=self.seg_t[:, b * G:(b + 1) * G].broadcast_to([P, G, P]),
                            op=mybir.AluOpType.is_equal,
                        )
                        self.batches[b] = (gt, oh)
                    return self.batches[b]

            lo = Stream("s2lo", s2lo_idx, s2lo_seg, S2LO, x_tab[0:LO, :], C2lo)
            streams2 = [lo]
            if LO < N:
                hi = Stream("s2hi", s2hi_idx, s2hi_seg, S2HI, x_tab[LO:N, :], C2hi)
                streams2.append(hi)

            # ---- stage A: Se[e] = sum_{pairs with edge e} X[v]
            for w in range(NW2):
                total_k = sum(int(s.counts[w]) for s in streams2)
                psum_w = winp.tile([P, P], f32, tag="win")
                k = 0
                for s in streams2:
                    for c in range(int(s.off[w]), int(s.off[w]) + int(s.counts[w])):
                        b, cl = divmod(c, G)
                        gt, oh = s.batch(b)
                        nc.tensor.matmul(
                            psum_w[:],
                            lhsT=oh[:, cl, :],
                            rhs=gt[:, cl, :],
                            start=(k == 0),
                            stop=(k == total_k - 1),
                        )
                        k += 1
                rows = min(P, EPC - w * P)
                st = sep.tile([P, P], bf16, tag="seflush")
                nc.vector.tensor_copy(out=st[:], in_=psum_w[:])
                nc.sync.dma_start(out=se_slice[w * P:w * P + rows, :], in_=st[:rows, :])

            # ---- AllGather Se
            ag = nc.gpsimd.collective_compute(
                "AllGather",
                mybir.AluOpType.bypass,
                replica_groups=[list(range(ncores))],
                ins=[se_slice[:]],
                outs=[se_full[:]],
            )

            # ---- stage B: T[v] = sum_{pairs with vertex v} Se[e] (transposed)
            s3 = Stream("s3", s3_idx, s3_seg, S3, se_full[:], C3)
            Tt = resident.tile([P, NW3 * P], f32, tag="Tt")
            for w in range(NW3):
                total_k = int(s3.counts[w])
                psum_w = winp.tile([P, P], f32, tag="win")
                for k, c in enumerate(range(int(s3.off[w]), int(s3.off[w]) + total_k)):
                    b, cl = divmod(c, G)
                    gt, oh = s3.batch(b)
                    nc.tensor.matmul(
                        psum_w[:],
                        lhsT=gt[:, cl, :],
                        rhs=oh[:, cl, :],
                        start=(k == 0),
                        stop=(k == total_k - 1),
                    )
                nc.vector.tensor_copy(out=Tt[:, w * P:(w + 1) * P], in_=psum_w[:])
            for inst in s3.gather_insts:
                add_dep_helper(inst.ins, ag.ins, reason="stage-B gathers read AllGathered Se")

            # ---- stage C: Zt = (1-a)*Xv^T + (a*X0 + affine)^T
            xd_t = resident.tile([D, VPC], f32, tag="xd")
            nc.sync.dma_start(xd_t[:], xd_d[:])
            x0h_t = resident.tile([D, VPC], f32, tag="x0h")
            nc.sync.dma_start(x0h_t[:], x0h_d[:])

            RT = 512
            for rt in range((VPC + RT - 1) // RT):
                s0 = rt * RT
                L = min(RT, VPC - s0)
                pz = zvp.tile([P, RT], f32, tag="zv")
                nc.tensor.matmul(pz[:, :L], lhsT=wa_t[:], rhs=xd_t[:, s0:s0 + L],
                                 start=True, stop=False)
                nc.tensor.matmul(pz[:, :L], lhsT=wt_t[:], rhs=Tt[:, s0:s0 + L],
                                 start=False, stop=True)
                nc.vector.tensor_add(out=x0h_t[:, s0:s0 + L], in0=pz[:, :L],
                                     in1=x0h_t[:, s0:s0 + L])

            # ---- stage D: out rows = Zt_tile^T @ W3 + b3
            for ot in range(NW3):
                s0 = ot * P
                L = min(P, VPC - s0)
                po = outp.tile([P, P], f32, tag="out")
                nc.tensor.matmul(po[:L, :], lhsT=x0h_t[:, s0:s0 + L], rhs=w3_t[:],
                                 start=True, stop=True)
                st = sep.tile([P, P], f32, tag="outflush")
                nc.vector.tensor_tensor(out=st[:L, :], in0=po[:L, :], in1=b3f_t[:L, :],
                                        op=mybir.AluOpType.add)
                nc.sync.dma_start(out=out_d[s0:s0 + L, :], in_=st[:L, :])

    return nc


# ---------------------------------------------------------------------------
# entry point
# ---------------------------------------------------------------------------

def _run(inputs, trace=False, M=25000, ncores=8, G=8, lo_split=32768):
    import sys
    if "/opt/trn_rl_repo" not in sys.path:
        sys.path.insert(0, "/opt/trn_rl_repo")
    from concourse.bass_utils import run_bass_kernel_spmd

    in_maps, meta = preprocess(**inputs, M=M, ncores=ncores, G=G, lo_split=lo_split)
    nc = build_program(meta)
    if not nc.is_finalized():
        nc.finalize()
    res = run_bass_kernel_spmd(nc, in_maps, list(range(ncores)), trace=trace)
    out = np.concatenate([np.asarray(res.results[i]["out"]) for i in range(ncores)], axis=0)
    return out, res


def kernel(**inputs):
    out, _ = _run(inputs)
    return out


# revision 15
# speedup vs baseline: 2.7246x; 1.0527x over previous
"""Trainium2 Bass kernel for nn_EquivSetConv (hypergraph message passing).

Reference computation:
    Xve = (X @ W1 + b1)[vertex]
    Xe  = segment_sum(Xve, edges, M)
    Xev = Xe[edges]
    H   = concat([X[vertex], Xev], -1) @ W2 + b2
    Xv  = segment_sum(H, vertex, N)
    out = ((1-a)*Xv + a*X0) @ W3 + b3

Algebraic restructure (A[v,e] = #incidence pairs (v,e)):
    Se  = A^T @ X                          (segmented sum of raw X rows per edge)
    Xe  = Se @ W1 + edeg x b1
    T   = A @ Se                           (segmented sum of Se rows per vertex)
    Xv  = deg . (X @ W2a) + T @ (W1 @ W2b) + deg x b2 + wdeg x (b1 @ W2b)
    out = ((1-a)Xv + a X0) @ W3 + b3

So the 800k-row dense matmul disappears; the kernel is two sparse
gather+segmented-sum stages plus small dense matmuls.

Sharding over 8 cores: stage A partitioned by edge range (each core owns
M/8 edges and all pairs incident to them -> computes its Se slice fully,
no cross-core reduction), one AllGather of Se, stage B partitioned by
vertex range (each core owns N/8 vertices -> computes its output rows
end to end). The only collective is the 1.6MB/rank AllGather.

Sparse stages on device: host sorts pairs by destination segment and
packs them into 128-pair chunks that are pure in a 128-wide segment
window.  For each chunk: dma_gather 128 source rows (pair p -> SBUF
partition p), build a one-hot [pair, segment-slot] matrix on DVE
(iota == seg), and accumulate with one PE matmul into the window's PSUM
tile.  Windows flush to SBUF/DRAM when complete.
"""

import numpy as np

P = 128
D = 128


# ---------------------------------------------------------------------------
# host-side preprocessing
# ---------------------------------------------------------------------------

def _pack_stream(seg_local, gidx, n_windows, chunk_counts):
    """Pack pairs (sorted by window) into window-pure 128-slot chunks.

    seg_local: [n] int, segment id LOCAL to the stream's window grid
               (seg_local // 128 = window, seg_local % 128 = slot)
    gidx:      [n] int, gather index of each pair
    chunk_counts: [n_windows] int, chunks allocated per window (shared
               across all cores so the program structure is identical).

    Returns (idx16, segf) flat arrays of length sum(chunk_counts)*128,
    pad slots have idx 0 / seg -1.
    """
    total_chunks = int(np.sum(chunk_counts))
    tot = total_chunks * P
    idx16 = np.zeros(tot, dtype=np.int16)
    segf = np.full(tot, -1.0, dtype=np.float32)
    if len(seg_local) == 0:
        return idx16, segf

    order = np.argsort(seg_local, kind="stable")
    seg_s = seg_local[order]
    gidx_s = gidx[order]
    win = seg_s // P

    # position of each pair: chunk_base[win]*128 + rank-within-window
    chunk_base = np.concatenate([[0], np.cumsum(chunk_counts)[:-1]])
    win_start = np.searchsorted(win, np.arange(n_windows), side="left")
    rank = np.arange(len(win)) - win_start[win]
    pos = chunk_base[win] * P + rank
    idx16[pos] = gidx_s.astype(np.int16)
    segf[pos] = (seg_s % P).astype(np.float32)
    return idx16, segf


def _wrap_idx(idx16, G):
    """Reshape a flat per-stream idx array into the dma_gather SBUF layout.

    Within each batch of G*128 indices, index i lives at
    [partition i%16, column i//16]; batches are side by side.
    Output [128, total_chunks*8] int16 (rows 0..15 replicated to 128).
    """
    nb = len(idx16) // (G * P)
    blocks = [idx16[b * G * P:(b + 1) * G * P].reshape(G * 8, 16).T for b in range(nb)]
    arr16 = np.hstack(blocks)  # [16, total_chunks*8]
    return np.tile(arr16, (8, 1)).astype(np.int16)


def _seg_tile(segf):
    """[total_chunks*128] -> [128, total_chunks]: pair (chunk c, part p)."""
    import ml_dtypes
    n_chunks = len(segf) // P
    return np.ascontiguousarray(segf.reshape(n_chunks, P).T).astype(ml_dtypes.bfloat16)


def _chunk_counts(windows_per_core, n_windows, G, min_one=True):
    """windows_per_core: list over cores of [n_windows] pair counts.
    Returns per-window chunk counts (max over cores), padded so the
    total is a multiple of G."""
    counts = np.zeros(n_windows, dtype=np.int64)
    for wc in windows_per_core:
        counts = np.maximum(counts, (wc + P - 1) // P)
    if min_one:
        counts = np.maximum(counts, 1)
    tot = int(counts.sum())
    rem = (-tot) % G
    counts[-1] += rem
    return counts


def preprocess(X, vertex, edges, X0, W1, b1, W2, b2, W3, b3,
               M=25000, ncores=8, G=8, lo_split=32768):
    """Build per-core input maps + compile-time metadata."""
    X = np.asarray(X, dtype=np.float32)
    X0 = np.asarray(X0, dtype=np.float32)
    vertex = np.asarray(vertex).astype(np.int64)
    edges = np.asarray(edges).astype(np.int64)
    W1 = np.asarray(W1, dtype=np.float32)
    b1 = np.asarray(b1, dtype=np.float32)
    W2 = np.asarray(W2, dtype=np.float32)
    b2 = np.asarray(b2, dtype=np.float32)
    W3 = np.asarray(W3, dtype=np.float32)
    b3 = np.asarray(b3, dtype=np.float32)

    N, Din = X.shape
    NNZ = len(vertex)
    Dout = W3.shape[1]
    assert Din == D and Dout == D

    LO = min(lo_split, N)
    EPC = M // ncores
    VPC = N // ncores
    assert M % ncores == 0 and N % ncores == 0
    NW2 = (EPC + P - 1) // P
    NW3 = (VPC + P - 1) // P

    alpha = 0.5
    W2a = W2[:D]
    W2b = W2[D:]
    deg = np.bincount(vertex, minlength=N).astype(np.float64)
    edeg = np.bincount(edges, minlength=M).astype(np.float64)
    wdeg = np.bincount(vertex, weights=edeg[edges], minlength=N)

    Wa = ((1.0 - alpha) * W2a).astype(np.float32)
    Wt = ((1.0 - alpha) * (W1.astype(np.float64) @ W2b.astype(np.float64))).astype(np.float32)
    b1w = (W2b.astype(np.float64).T @ b1.astype(np.float64))  # b1 @ W2b
    b3_full = np.tile(b3[None, :], (P, 1)).astype(np.float32)

    core_edge = edges // EPC
    core_vert = vertex // VPC

    H = (EPC // 2 // P) * P if EPC >= 2 * P else max(EPC // 2, 1)

    # ---- per-core pair lists
    s2lo_w, s2hi_w, s3a_w, s3b_w = [], [], [], []
    s2lo_pairs, s2hi_pairs, s3a_pairs, s3b_pairs = [], [], [], []
    for i in range(ncores):
        sel = np.nonzero(core_edge == i)[0]
        pv = vertex[sel]
        pe = edges[sel] - i * EPC
        mlo = pv < LO
        for store_w, store_p, v, e in (
            (s2lo_w, s2lo_pairs, pv[mlo], pe[mlo]),
            (s2hi_w, s2hi_pairs, pv[~mlo] - LO, pe[~mlo]),
        ):
            store_w.append(np.bincount(e // P, minlength=NW2))
            store_p.append((e, v))
        sel = np.nonzero(core_vert == i)[0]
        pe = edges[sel]
        pvl = vertex[sel] - i * VPC
        el = pe % EPC
        er = pe // EPC
        mha = el < H
        # remapped gather indices into the two half-AllGather layouts
        ia = er[mha] * H + el[mha]
        ib = er[~mha] * (EPC - H) + (el[~mha] - H)
        s3a_w.append(np.bincount(pvl[mha] // P, minlength=NW3))
        s3b_w.append(np.bincount(pvl[~mha] // P, minlength=NW3))
        s3a_pairs.append((pvl[mha], ia))
        s3b_pairs.append((pvl[~mha], ib))

    C2lo = _chunk_counts(s2lo_w, NW2, G)
    C2hi = _chunk_counts(s2hi_w, NW2, G)
    C3a = _chunk_counts(s3a_w, NW3, G)
    C3b = _chunk_counts(s3b_w, NW3, G, min_one=False)
    S2LO, S2HI = int(C2lo.sum()), int(C2hi.sum())
    S3A, S3B = int(C3a.sum()), int(C3b.sum())

    iota = np.tile(np.arange(P, dtype=np.float32), G)[None, :].repeat(P, axis=0)
    iota = np.ascontiguousarray(iota)

    import ml_dtypes
    X_bf16 = X.astype(ml_dtypes.bfloat16)
    iota = iota.astype(ml_dtypes.bfloat16)
    W3h = W3.astype(ml_dtypes.bfloat16)

    in_maps = []
    for i in range(ncores):
        e, v = s2lo_pairs[i]
        lo_idx, lo_seg = _pack_stream(e, v, NW2, C2lo)
        e, v = s2hi_pairs[i]
        hi_idx, hi_seg = _pack_stream(e, v, NW2, C2hi)
        pvl, ia = s3a_pairs[i]
        s3a_idx, s3a_seg = _pack_stream(pvl, ia, NW3, C3a)
        pvl, ib = s3b_pairs[i]
        s3b_idx, s3b_seg = _pack_stream(pvl, ib, NW3, C3b)

        sl = slice(i * VPC, (i + 1) * VPC)
        xd_t = np.ascontiguousarray((X[sl].astype(np.float64) * deg[sl, None]).T).astype(np.float32)
        x0h = alpha * X0[sl].astype(np.float64).T \
            + (1.0 - alpha) * (np.outer(b2, deg[sl]) + np.outer(b1w, wdeg[sl]))
        x0h_t = np.ascontiguousarray(x0h).astype(np.float32)

        in_maps.append({
            "x_tab": X_bf16,
            "s2lo_idx": _wrap_idx(lo_idx, G), "s2lo_seg": _seg_tile(lo_seg),
            "s2hi_idx": _wrap_idx(hi_idx, G), "s2hi_seg": _seg_tile(hi_seg),
            "s3a_idx": _wrap_idx(s3a_idx, G), "s3a_seg": _seg_tile(s3a_seg),
            "s3b_idx": _wrap_idx(s3b_idx, G), "s3b_seg": _seg_tile(s3b_seg),
            "iota": iota,
            "xd_t": xd_t,
            "x0h_t": x0h_t,
            "wa": Wa, "wt": Wt, "w3": W3h, "b3f": b3_full,
        })

    meta = dict(N=N, M=M, NNZ=NNZ, ncores=ncores, G=G, LO=LO, H=H,
                EPC=EPC, VPC=VPC, NW2=NW2, NW3=NW3,
                C2lo=C2lo.tolist(), C2hi=C2hi.tolist(),
                C3a=C3a.tolist(), C3b=C3b.tolist(),
                S2LO=S2LO, S2HI=S2HI, S3A=S3A, S3B=S3B)
    return in_maps, meta


# ---------------------------------------------------------------------------
# device program
# ---------------------------------------------------------------------------

def build_program(meta):
    import concourse.bacc as bacc
    import concourse.bass as bass  # noqa: F401
    import concourse.mybir as mybir
    import concourse.tile as tile
    from concourse._compat import get_trn_type
    from concourse import library_config
    from concourse.tile_rust import add_dep_helper

    f32 = mybir.dt.float32
    bf16 = mybir.dt.bfloat16
    i16 = mybir.dt.int16

    ncores = meta["ncores"]
    G = meta["G"]
    N, M = meta["N"], meta["M"]
    LO = meta["LO"]
    EPC, VPC = meta["EPC"], meta["VPC"]
    NW2, NW3 = meta["NW2"], meta["NW3"]
    C2lo, C2hi = meta["C2lo"], meta["C2hi"]
    C3a, C3b = meta["C3a"], meta["C3b"]
    S2LO, S2HI = meta["S2LO"], meta["S2HI"]
    S3A, S3B = meta["S3A"], meta["S3B"]
    H = meta["H"]
    GP = G * P

    nc = bacc.Bacc(get_trn_type() or "TRN2", num_devices=ncores, num_swdge_queues=4)

    x_tab = nc.declare_dram_parameter("x_tab", [N, D], bf16, isOutput=False)
    s2lo_idx = nc.declare_dram_parameter("s2lo_idx", [P, S2LO * 8], i16, isOutput=False)
    s2lo_seg = nc.declare_dram_parameter("s2lo_seg", [P, S2LO], bf16, isOutput=False)
    s2hi_idx = nc.declare_dram_parameter("s2hi_idx", [P, S2HI * 8], i16, isOutput=False)
    s2hi_seg = nc.declare_dram_parameter("s2hi_seg", [P, S2HI], bf16, isOutput=False)
    s3a_idx = nc.declare_dram_parameter("s3a_idx", [P, S3A * 8], i16, isOutput=False)
    s3a_seg = nc.declare_dram_parameter("s3a_seg", [P, S3A], bf16, isOutput=False)
    s3b_idx = nc.declare_dram_parameter("s3b_idx", [P, S3B * 8], i16, isOutput=False)
    s3b_seg = nc.declare_dram_parameter("s3b_seg", [P, S3B], bf16, isOutput=False)
    iota_d = nc.declare_dram_parameter("iota", [P, GP], bf16, isOutput=False)
    xd_d = nc.declare_dram_parameter("xd_t", [D, VPC], f32, isOutput=False)
    x0h_d = nc.declare_dram_parameter("x0h_t", [D, VPC], f32, isOutput=False)
    wa_d = nc.declare_dram_parameter("wa", [D, D], f32, isOutput=False)
    wt_d = nc.declare_dram_parameter("wt", [D, D], f32, isOutput=False)
    w3_d = nc.declare_dram_parameter("w3", [D, D], bf16, isOutput=False)
    b3f_d = nc.declare_dram_parameter("b3f", [P, D], f32, isOutput=False)
    out_d = nc.declare_dram_parameter("out", [VPC, D], f32, isOutput=True)

    se_slice = nc.dram_tensor("se_slice", [EPC, D], bf16)
    se_h1 = nc.dram_tensor("se_h1", [ncores * H, D], bf16, addr_space="Shared")
    se_h2 = nc.dram_tensor("se_h2", [ncores * (EPC - H), D], bf16, addr_space="Shared")

    with tile.TileContext(nc) as tc:
        with (
            tc.tile_pool(name="consts", bufs=1) as consts,
            tc.tile_pool(name="resident", bufs=1) as resident,
            tc.tile_pool(name="gat", bufs=4) as gat,
            tc.tile_pool(name="ohp", bufs=4) as ohp,
            tc.tile_pool(name="sep", bufs=3) as sep,
            tc.tile_pool(name="winp", bufs=3, space="PSUM") as winp,
            tc.tile_pool(name="zvp", bufs=2, space="PSUM") as zvp,
            tc.tile_pool(name="outp", bufs=2, space="PSUM") as outp,
        ):
            # ---- resident loads
            iota_t = consts.tile([P, G, P], bf16)
            nc.sync.dma_start(iota_t[:], iota_d[:].rearrange("p (g q) -> p g q", q=P))
            wa_t = consts.tile([D, D], f32)
            nc.sync.dma_start(wa_t[:], wa_d[:])
            wt_t = consts.tile([D, D], f32)
            nc.sync.dma_start(wt_t[:], wt_d[:])
            w3_t = consts.tile([D, D], bf16)
            nc.sync.dma_start(w3_t[:], w3_d[:])
            b3f_t = consts.tile([P, D], f32)
            nc.sync.dma_start(b3f_t[:], b3f_d[:])

            nc.gpsimd.load_library(library_config.mlp)
            npairs_reg = nc.gpsimd.to_reg(GP)
            qctr = [0]

            class Stream:
                def __init__(self, name, idx_d, seg_d, n_chunks, table_ap, counts):
                    self.name = name
                    self.counts = counts
                    self.off = np.concatenate([[0], np.cumsum(counts)[:-1]]).astype(int)
                    self.table_ap = table_ap
                    self.idx_t = resident.tile([P, n_chunks * 8], i16, tag=f"idx_{name}")
                    nc.sync.dma_start(self.idx_t[:], idx_d[:])
                    self.seg_t = resident.tile([P, n_chunks], bf16, tag=f"seg_{name}")
                    nc.sync.dma_start(self.seg_t[:], seg_d[:])
                    self.batches = {}
                    self.gather_insts = []

                def batch(self, b):
                    if b not in self.batches:
                        gt = gat.tile([P, G, D], bf16, tag="gat")
                        inst = nc.gpsimd.dma_gather(
                            gt[:],
                            self.table_ap,
                            self.idx_t[:, b * G * 8:(b + 1) * G * 8],
                            GP,
                            npairs_reg,
                            D,
                            queue_num=qctr[0] % 4,
                        )
                        qctr[0] += 1
                        self.gather_insts.append(inst)
                        oh = ohp.tile([P, G, P], bf16, tag="oh")
                        nc.vector.tensor_tensor(
                            out=oh[:],
                            in0=iota_t[:],
                            in1=self.seg_t[:, b * G:(b + 1) * G].broadcast_to([P, G, P]),
                            op=mybir.AluOpType.is_equal,
                        )
                        self.batches[b] = (gt, oh)
                    return self.batches[b]

            lo = Stream("s2lo", s2lo_idx, s2lo_seg, S2LO, x_tab[0:LO, :], C2lo)
            streams2 = [lo]
            if LO < N:
                hi = Stream("s2hi", s2hi_idx, s2hi_seg, S2HI, x_tab[LO:N, :], C2hi)
                streams2.append(hi)

            # ---- stage A: Se[e] = sum_{pairs with edge e} X[v]
            flushes_h1 = []
            flushes_h2 = []
            ag1 = ag2 = None
            w_ag1 = (H - 1) // P  # AG1 after this window's flush
            for w in range(NW2):
                total_k = sum(int(s.counts[w]) for s in streams2)
                psum_w = winp.tile([P, P], f32, tag="win")
                k = 0
                for s in streams2:
                    for c in range(int(s.off[w]), int(s.off[w]) + int(s.counts[w])):
                        b, cl = divmod(c, G)
                        gt, oh = s.batch(b)
                        nc.tensor.matmul(
                            psum_w[:],
                            lhsT=oh[:, cl, :],
                            rhs=gt[:, cl, :],
                            start=(k == 0),
                            stop=(k == total_k - 1),
                        )
                        k += 1
                rows = min(P, EPC - w * P)
                st = sep.tile([P, P], bf16, tag="seflush")
                nc.vector.tensor_copy(out=st[:], in_=psum_w[:])
                fl = nc.sync.dma_start(out=se_slice[w * P:w * P + rows, :], in_=st[:rows, :])
                if w * P < H:
                    flushes_h1.append(fl)
                if w * P + rows > H:
                    flushes_h2.append(fl)
                if w == w_ag1:
                    ag1 = nc.gpsimd.collective_compute(
                        "AllGather", mybir.AluOpType.bypass,
                        replica_groups=[list(range(ncores))],
                        ins=[se_slice[0:H, :]], outs=[se_h1[:]])
                    for f in flushes_h1:
                        add_dep_helper(ag1.ins, f.ins, reason="AG1 reads se_slice[:H]")
            ag2 = nc.gpsimd.collective_compute(
                "AllGather", mybir.AluOpType.bypass,
                replica_groups=[list(range(ncores))],
                ins=[se_slice[H:EPC, :]], outs=[se_h2[:]])
            for f in flushes_h2:
                add_dep_helper(ag2.ins, f.ins, reason="AG2 reads se_slice[H:]")

            # ---- stage B: T[v] = sum_{pairs with vertex v} Se[e] (transposed)
            # pass A over half-1 edges (after AG1), pass B adds half-2 (after AG2),
            # with the dense tail (stages C/D) interleaved as windows finalize.
            s3a = Stream("s3a", s3a_idx, s3a_seg, S3A, se_h1[:], C3a)
            s3b = Stream("s3b", s3b_idx, s3b_seg, S3B, se_h2[:], C3b)
            Tt = resident.tile([P, NW3 * P], f32, tag="Tt")
            xd_t = resident.tile([D, VPC], f32, tag="xd")
            nc.sync.dma_start(xd_t[:], xd_d[:])
            x0h_t = resident.tile([D, VPC], f32, tag="x0h")
            nc.sync.dma_start(x0h_t[:], x0h_d[:])
            zt_t = resident.tile([D, VPC], bf16, tag="zt")

            for w in range(NW3):
                total_k = int(s3a.counts[w])
                psum_w = winp.tile([P, P], f32, tag="win")
                for k, c in enumerate(range(int(s3a.off[w]), int(s3a.off[w]) + total_k)):
                    b, cl = divmod(c, G)
                    gt, oh = s3a.batch(b)
                    nc.tensor.matmul(
                        psum_w[:],
                        lhsT=gt[:, cl, :],
                        rhs=oh[:, cl, :],
                        start=(k == 0),
                        stop=(k == total_k - 1),
                    )
                nc.vector.tensor_copy(out=Tt[:, w * P:(w + 1) * P], in_=psum_w[:])

            RT = 512

            def emit_c_tile(rt):
                s0 = rt * RT
                L = min(RT, VPC - s0)
                pz = zvp.tile([P, RT], f32, tag="zv")
                nc.tensor.matmul(pz[:, :L], lhsT=wa_t[:], rhs=xd_t[:, s0:s0 + L],
                                 start=True, stop=False)
                nc.tensor.matmul(pz[:, :L], lhsT=wt_t[:], rhs=Tt[:, s0:s0 + L],
                                 start=False, stop=True)
                nc.vector.tensor_add(out=zt_t[:, s0:s0 + L], in0=pz[:, :L],
                                     in1=x0h_t[:, s0:s0 + L])
                for ot in range(s0 // P, (s0 + L + P - 1) // P):
                    o0 = ot * P
                    Lo = min(P, VPC - o0)
                    po = outp.tile([P, P], f32, tag="out")
                    nc.tensor.matmul(po[:Lo, :], lhsT=zt_t[:, o0:o0 + Lo], rhs=w3_t[:],
                                     start=True, stop=True)
                    st = sep.tile([P, P], f32, tag="outflush")
                    nc.vector.tensor_tensor(out=st[:Lo, :], in0=po[:Lo, :],
                                            in1=b3f_t[:Lo, :], op=mybir.AluOpType.add)
                    nc.sync.dma_start(out=out_d[o0:o0 + Lo, :], in_=st[:Lo, :])

            n_ctiles = (VPC + RT - 1) // RT
            done_c = 0
            for w in range(NW3):
                total_k = int(s3b.counts[w])
                if total_k > 0:
                    psum_w = winp.tile([P, P], f32, tag="win")
                    for k, c in enumerate(range(int(s3b.off[w]), int(s3b.off[w]) + total_k)):
                        b, cl = divmod(c, G)
                        gt, oh = s3b.batch(b)
                        nc.tensor.matmul(
                            psum_w[:],
                            lhsT=gt[:, cl, :],
                            rhs=oh[:, cl, :],
                            start=(k == 0),
                            stop=(k == total_k - 1),
                        )
                    nc.vector.tensor_add(out=Tt[:, w * P:(w + 1) * P],
                                         in0=Tt[:, w * P:(w + 1) * P], in1=psum_w[:])
                # emit any C tiles fully covered by finalized windows
                while done_c < n_ctiles and (done_c + 1) * RT <= (w + 1) * P:
                    emit_c_tile(done_c)
                    done_c += 1
            while done_c < n_ctiles:
                emit_c_tile(done_c)
                done_c += 1

            for inst in s3a.gather_insts:
                add_dep_helper(inst.ins, ag1.ins, reason="pass-A gathers read se_h1")
            for inst in s3b.gather_insts:
                add_dep_helper(inst.ins, ag2.ins, reason="pass-B gathers read se_h2")

    return nc


# ---------------------------------------------------------------------------
# entry point
# ---------------------------------------------------------------------------

def _run(inputs, trace=False, M=25000, ncores=8, G=8, lo_split=32768):
    import sys
    if "/opt/trn_rl_repo" not in sys.path:
        sys.path.insert(0, "/opt/trn_rl_repo")
    from concourse.bass_utils import run_bass_kernel_spmd

    in_maps, meta = preprocess(**inputs, M=M, ncores=ncores, G=G, lo_split=lo_split)
    nc = build_program(meta)
    if not nc.is_finalized():
        nc.finalize()
    res = run_bass_kernel_spmd(nc, in_maps, list(range(ncores)), trace=trace)
    out = np.concatenate([np.asarray(res.results[i]["out"]) for i in range(ncores)], axis=0)
    return out, res


def kernel(**inputs):
    out, _ = _run(inputs)
    return out


# revision 16
# speedup vs baseline: 3.2523x; 1.1937x over previous
"""Trainium2 Bass kernel for nn_EquivSetConv (hypergraph message passing).

Reference computation:
    Xve = (X @ W1 + b1)[vertex]
    Xe  = segment_sum(Xve, edges, M)
    Xev = Xe[edges]
    H   = concat([X[vertex], Xev], -1) @ W2 + b2
    Xv  = segment_sum(H, vertex, N)
    out = ((1-a)*Xv + a*X0) @ W3 + b3

Algebraic restructure (A[v,e] = #incidence pairs (v,e)):
    Se  = A^T @ X                          (segmented sum of raw X rows per edge)
    Xe  = Se @ W1 + edeg x b1
    T   = A @ Se                           (segmented sum of Se rows per vertex)
    Xv  = deg . (X @ W2a) + T @ (W1 @ W2b) + deg x b2 + wdeg x (b1 @ W2b)
    out = ((1-a)Xv + a X0) @ W3 + b3

So the 800k-row dense matmul disappears; the kernel is two sparse
gather+segmented-sum stages plus small dense matmuls.

Sharding over 8 cores: stage A partitioned by edge range (each core owns
M/8 edges and all pairs incident to them -> computes its Se slice fully,
no cross-core reduction), one AllGather of Se, stage B partitioned by
vertex range (each core owns N/8 vertices -> computes its output rows
end to end). The only collective is the 1.6MB/rank AllGather.

Sparse stages on device: host sorts pairs by destination segment and
packs them into 128-pair chunks that are pure in a 128-wide segment
window.  For each chunk: dma_gather 128 source rows (pair p -> SBUF
partition p), build a one-hot [pair, segment-slot] matrix on DVE
(iota == seg), and accumulate with one PE matmul into the window's PSUM
tile.  Windows flush to SBUF/DRAM when complete.
"""

import numpy as np

P = 128
D = 128


# ---------------------------------------------------------------------------
# host-side preprocessing
# ---------------------------------------------------------------------------

def _pack_stream(seg_local, gidx, n_windows, chunk_counts):
    """Pack pairs (sorted by window) into window-pure 128-slot chunks.

    seg_local: [n] int, segment id LOCAL to the stream's window grid
               (seg_local // 128 = window, seg_local % 128 = slot)
    gidx:      [n] int, gather index of each pair
    chunk_counts: [n_windows] int, chunks allocated per window (shared
               across all cores so the program structure is identical).

    Returns (idx16, segf) flat arrays of length sum(chunk_counts)*128,
    pad slots have idx 0 / seg -1.
    """
    total_chunks = int(np.sum(chunk_counts))
    tot = total_chunks * P
    idx16 = np.zeros(tot, dtype=np.int16)
    segf = np.full(tot, -1.0, dtype=np.float32)
    if len(seg_local) == 0:
        return idx16, segf

    order = np.argsort(seg_local, kind="stable")
    seg_s = seg_local[order]
    gidx_s = gidx[order]
    win = seg_s // P

    # position of each pair: chunk_base[win]*128 + rank-within-window
    chunk_base = np.concatenate([[0], np.cumsum(chunk_counts)[:-1]])
    win_start = np.searchsorted(win, np.arange(n_windows), side="left")
    rank = np.arange(len(win)) - win_start[win]
    pos = chunk_base[win] * P + rank
    idx16[pos] = gidx_s.astype(np.int16)
    segf[pos] = (seg_s % P).astype(np.float32)
    return idx16, segf


def _wrap_idx(idx16, G):
    """Reshape a flat per-stream idx array into the dma_gather SBUF layout.

    Within each batch of G*128 indices, index i lives at
    [partition i%16, column i//16]; batches are side by side.
    Output [128, total_chunks*8] int16 (rows 0..15 replicated to 128).
    """
    nb = len(idx16) // (G * P)
    blocks = [idx16[b * G * P:(b + 1) * G * P].reshape(G * 8, 16).T for b in range(nb)]
    arr16 = np.hstack(blocks)  # [16, total_chunks*8]
    return np.tile(arr16, (8, 1)).astype(np.int16)


def _seg_tile(segf):
    """[total_chunks*128] -> [128, total_chunks]: pair (chunk c, part p)."""
    import ml_dtypes
    n_chunks = len(segf) // P
    return np.ascontiguousarray(segf.reshape(n_chunks, P).T).astype(ml_dtypes.bfloat16)


def _chunk_counts(windows_per_core, n_windows, G, min_one=True):
    """windows_per_core: list over cores of [n_windows] pair counts.
    Returns per-window chunk counts (max over cores), padded so the
    total is a multiple of G."""
    counts = np.zeros(n_windows, dtype=np.int64)
    for wc in windows_per_core:
        counts = np.maximum(counts, (wc + P - 1) // P)
    if min_one:
        counts = np.maximum(counts, 1)
    tot = int(counts.sum())
    rem = (-tot) % G
    counts[-1] += rem
    return counts


def preprocess(X, vertex, edges, X0, W1, b1, W2, b2, W3, b3,
               M=25000, ncores=8, G=8, lo_split=32768):
    """Build per-core input maps + compile-time metadata."""
    X = np.asarray(X, dtype=np.float32)
    X0 = np.asarray(X0, dtype=np.float32)
    vertex = np.asarray(vertex).astype(np.int64)
    edges = np.asarray(edges).astype(np.int64)
    W1 = np.asarray(W1, dtype=np.float32)
    b1 = np.asarray(b1, dtype=np.float32)
    W2 = np.asarray(W2, dtype=np.float32)
    b2 = np.asarray(b2, dtype=np.float32)
    W3 = np.asarray(W3, dtype=np.float32)
    b3 = np.asarray(b3, dtype=np.float32)

    N, Din = X.shape
    NNZ = len(vertex)
    Dout = W3.shape[1]
    assert Din == D and Dout == D

    LO = min(lo_split, N)
    EPC = M // ncores
    VPC = N // ncores
    assert M % ncores == 0 and N % ncores == 0
    NW2 = (EPC + P - 1) // P
    NW3 = (VPC + P - 1) // P

    alpha = 0.5
    W2a = W2[:D]
    W2b = W2[D:]
    deg = np.bincount(vertex, minlength=N).astype(np.float64)
    edeg = np.bincount(edges, minlength=M).astype(np.float64)
    wdeg = np.bincount(vertex, weights=edeg[edges], minlength=N)

    Wa = ((1.0 - alpha) * W2a).astype(np.float32)
    Wt = ((1.0 - alpha) * (W1.astype(np.float64) @ W2b.astype(np.float64))).astype(np.float32)
    b1w = (W2b.astype(np.float64).T @ b1.astype(np.float64))  # b1 @ W2b
    b3_full = np.tile(b3[None, :], (P, 1)).astype(np.float32)

    core_edge = edges // EPC
    core_vert = vertex // VPC

    H = (EPC // 2 // P) * P if EPC >= 2 * P else max(EPC // 2, 1)

    # ---- per-core pair lists
    s2lo_w, s2hi_w, s3a_w, s3b_w = [], [], [], []
    s2lo_pairs, s2hi_pairs, s3a_pairs, s3b_pairs = [], [], [], []
    for i in range(ncores):
        sel = np.nonzero(core_edge == i)[0]
        pv = vertex[sel]
        pe = edges[sel] - i * EPC
        mlo = pv < LO
        for store_w, store_p, v, e in (
            (s2lo_w, s2lo_pairs, pv[mlo], pe[mlo]),
            (s2hi_w, s2hi_pairs, pv[~mlo] - LO, pe[~mlo]),
        ):
            store_w.append(np.bincount(e // P, minlength=NW2))
            store_p.append((e, v))
        sel = np.nonzero(core_vert == i)[0]
        pe = edges[sel]
        pvl = vertex[sel] - i * VPC
        el = pe % EPC
        er = pe // EPC
        mha = el < H
        # remapped gather indices into the two half-AllGather layouts
        ia = er[mha] * H + el[mha]
        ib = er[~mha] * (EPC - H) + (el[~mha] - H)
        s3a_w.append(np.bincount(pvl[mha] // P, minlength=NW3))
        s3b_w.append(np.bincount(pvl[~mha] // P, minlength=NW3))
        s3a_pairs.append((pvl[mha], ia))
        s3b_pairs.append((pvl[~mha], ib))

    C2lo = _chunk_counts(s2lo_w, NW2, G)
    C2hi = _chunk_counts(s2hi_w, NW2, G)
    C3a = _chunk_counts(s3a_w, NW3, G)
    C3b = _chunk_counts(s3b_w, NW3, G, min_one=False)
    S2LO, S2HI = int(C2lo.sum()), int(C2hi.sum())
    S3A, S3B = int(C3a.sum()), int(C3b.sum())

    iota = np.tile(np.arange(P, dtype=np.float32), G)[None, :].repeat(P, axis=0)
    iota = np.ascontiguousarray(iota)

    import ml_dtypes
    X_bf16 = X.astype(ml_dtypes.bfloat16)
    iota = iota.astype(ml_dtypes.bfloat16)
    W3h = W3.astype(ml_dtypes.bfloat16)

    in_maps = []
    for i in range(ncores):
        e, v = s2lo_pairs[i]
        lo_idx, lo_seg = _pack_stream(e, v, NW2, C2lo)
        e, v = s2hi_pairs[i]
        hi_idx, hi_seg = _pack_stream(e, v, NW2, C2hi)
        pvl, ia = s3a_pairs[i]
        s3a_idx, s3a_seg = _pack_stream(pvl, ia, NW3, C3a)
        pvl, ib = s3b_pairs[i]
        s3b_idx, s3b_seg = _pack_stream(pvl, ib, NW3, C3b)

        sl = slice(i * VPC, (i + 1) * VPC)
        xd_t = np.ascontiguousarray((X[sl].astype(np.float64) * deg[sl, None]).T).astype(np.float32)
        x0h = alpha * X0[sl].astype(np.float64).T \
            + (1.0 - alpha) * (np.outer(b2, deg[sl]) + np.outer(b1w, wdeg[sl]))
        x0h_t = np.ascontiguousarray(x0h).astype(np.float32)

        in_maps.append({
            "x_tab": X_bf16,
            "s2lo_idx": _wrap_idx(lo_idx, G), "s2lo_seg": _seg_tile(lo_seg),
            "s2hi_idx": _wrap_idx(hi_idx, G), "s2hi_seg": _seg_tile(hi_seg),
            "s3a_idx": _wrap_idx(s3a_idx, G), "s3a_seg": _seg_tile(s3a_seg),
            "s3b_idx": _wrap_idx(s3b_idx, G), "s3b_seg": _seg_tile(s3b_seg),
            "iota": iota,
            "xd_t": xd_t,
            "x0h_t": x0h_t,
            "wa": Wa, "wt": Wt, "w3": W3h, "b3f": b3_full,
        })

    meta = dict(N=N, M=M, NNZ=NNZ, ncores=ncores, G=G, LO=LO, H=H,
                EPC=EPC, VPC=VPC, NW2=NW2, NW3=NW3,
                C2lo=C2lo.tolist(), C2hi=C2hi.tolist(),
                C3a=C3a.tolist(), C3b=C3b.tolist(),
                S2LO=S2LO, S2HI=S2HI, S3A=S3A, S3B=S3B)
    return in_maps, meta


# ---------------------------------------------------------------------------
# device program
# ---------------------------------------------------------------------------

def build_program(meta):
    import concourse.bacc as bacc
    import concourse.bass as bass  # noqa: F401
    import concourse.mybir as mybir
    import concourse.tile as tile
    from concourse._compat import get_trn_type
    from concourse import library_config
    from concourse.tile_rust import add_dep_helper

    f32 = mybir.dt.float32
    bf16 = mybir.dt.bfloat16
    i16 = mybir.dt.int16

    ncores = meta["ncores"]
    G = meta["G"]
    N, M = meta["N"], meta["M"]
    LO = meta["LO"]
    EPC, VPC = meta["EPC"], meta["VPC"]
    NW2, NW3 = meta["NW2"], meta["NW3"]
    C2lo, C2hi = meta["C2lo"], meta["C2hi"]
    C3a, C3b = meta["C3a"], meta["C3b"]
    S2LO, S2HI = meta["S2LO"], meta["S2HI"]
    S3A, S3B = meta["S3A"], meta["S3B"]
    H = meta["H"]
    GP = G * P

    nc = bacc.Bacc(get_trn_type() or "TRN2", num_devices=ncores, num_swdge_queues=4)

    x_tab = nc.declare_dram_parameter("x_tab", [N, D], bf16, isOutput=False)
    s2lo_idx = nc.declare_dram_parameter("s2lo_idx", [P, S2LO * 8], i16, isOutput=False)
    s2lo_seg = nc.declare_dram_parameter("s2lo_seg", [P, S2LO], bf16, isOutput=False)
    s2hi_idx = nc.declare_dram_parameter("s2hi_idx", [P, S2HI * 8], i16, isOutput=False)
    s2hi_seg = nc.declare_dram_parameter("s2hi_seg", [P, S2HI], bf16, isOutput=False)
    s3a_idx = nc.declare_dram_parameter("s3a_idx", [P, S3A * 8], i16, isOutput=False)
    s3a_seg = nc.declare_dram_parameter("s3a_seg", [P, S3A], bf16, isOutput=False)
    s3b_idx = nc.declare_dram_parameter("s3b_idx", [P, S3B * 8], i16, isOutput=False)
    s3b_seg = nc.declare_dram_parameter("s3b_seg", [P, S3B], bf16, isOutput=False)
    iota_d = nc.declare_dram_parameter("iota", [P, GP], bf16, isOutput=False)
    xd_d = nc.declare_dram_parameter("xd_t", [D, VPC], f32, isOutput=False)
    x0h_d = nc.declare_dram_parameter("x0h_t", [D, VPC], f32, isOutput=False)
    wa_d = nc.declare_dram_parameter("wa", [D, D], f32, isOutput=False)
    wt_d = nc.declare_dram_parameter("wt", [D, D], f32, isOutput=False)
    w3_d = nc.declare_dram_parameter("w3", [D, D], bf16, isOutput=False)
    b3f_d = nc.declare_dram_parameter("b3f", [P, D], f32, isOutput=False)
    out_d = nc.declare_dram_parameter("out", [VPC, D], f32, isOutput=True)

    se_slice = nc.dram_tensor("se_slice", [EPC, D], bf16)
    se_h1 = nc.dram_tensor("se_h1", [ncores * H, D], bf16, addr_space="Shared")
    se_h2 = nc.dram_tensor("se_h2", [ncores * (EPC - H), D], bf16, addr_space="Shared")

    with tile.TileContext(nc) as tc:
        with (
            tc.tile_pool(name="consts", bufs=1) as consts,
            tc.tile_pool(name="resident", bufs=1) as resident,
            tc.tile_pool(name="gat", bufs=6) as gat,
            tc.tile_pool(name="ohp", bufs=6) as ohp,
            tc.tile_pool(name="sep", bufs=3) as sep,
            tc.tile_pool(name="winp", bufs=4, space="PSUM") as winp,
            tc.tile_pool(name="zvp", bufs=2, space="PSUM") as zvp,
            tc.tile_pool(name="outp", bufs=2, space="PSUM") as outp,
        ):
            # ---- resident loads
            iota_t = consts.tile([P, G, P], bf16)
            nc.sync.dma_start(iota_t[:], iota_d[:].rearrange("p (g q) -> p g q", q=P))
            wa_t = consts.tile([D, D], f32)
            nc.sync.dma_start(wa_t[:], wa_d[:])
            wt_t = consts.tile([D, D], f32)
            nc.sync.dma_start(wt_t[:], wt_d[:])
            w3_t = consts.tile([D, D], bf16)
            nc.sync.dma_start(w3_t[:], w3_d[:])
            b3f_t = consts.tile([P, D], f32)
            nc.sync.dma_start(b3f_t[:], b3f_d[:])

            nc.gpsimd.load_library(library_config.mlp)
            npairs_reg = nc.gpsimd.to_reg(GP)
            qctr = [0]

            class Stream:
                def __init__(self, name, idx_d, seg_d, n_chunks, table_ap, counts):
                    self.name = name
                    self.counts = counts
                    self.off = np.concatenate([[0], np.cumsum(counts)[:-1]]).astype(int)
                    self.table_ap = table_ap
                    self.idx_t = resident.tile([P, n_chunks * 8], i16, tag=f"idx_{name}")
                    nc.sync.dma_start(self.idx_t[:], idx_d[:])
                    self.seg_t = resident.tile([P, n_chunks], bf16, tag=f"seg_{name}")
                    nc.sync.dma_start(self.seg_t[:], seg_d[:])
                    self.batches = {}
                    self.gather_insts = []

                def batch(self, b):
                    if b not in self.batches:
                        gt = gat.tile([P, G, D], bf16, tag="gat")
                        inst = nc.gpsimd.dma_gather(
                            gt[:],
                            self.table_ap,
                            self.idx_t[:, b * G * 8:(b + 1) * G * 8],
                            GP,
                            npairs_reg,
                            D,
                            queue_num=qctr[0] % 4,
                        )
                        qctr[0] += 1
                        self.gather_insts.append(inst)
                        oh = ohp.tile([P, G, P], bf16, tag="oh")
                        nc.vector.tensor_tensor(
                            out=oh[:],
                            in0=iota_t[:],
                            in1=self.seg_t[:, b * G:(b + 1) * G].broadcast_to([P, G, P]),
                            op=mybir.AluOpType.is_equal,
                        )
                        self.batches[b] = (gt, oh)
                    return self.batches[b]

            lo = Stream("s2lo", s2lo_idx, s2lo_seg, S2LO, x_tab[0:LO, :], C2lo)
            streams2 = [lo]
            if LO < N:
                hi = Stream("s2hi", s2hi_idx, s2hi_seg, S2HI, x_tab[LO:N, :], C2hi)
                streams2.append(hi)

            # ---- stage A: Se[e] = sum_{pairs with edge e} X[v]
            flushes_h1 = []
            flushes_h2 = []
            ag1 = ag2 = None
            w_ag1 = (H - 1) // P  # AG1 after this window's flush
            for w in range(NW2):
                total_k = sum(int(s.counts[w]) for s in streams2)
                psum_w = winp.tile([P, P], f32, tag="win")
                k = 0
                for s in streams2:
                    for c in range(int(s.off[w]), int(s.off[w]) + int(s.counts[w])):
                        b, cl = divmod(c, G)
                        gt, oh = s.batch(b)
                        nc.tensor.matmul(
                            psum_w[:],
                            lhsT=oh[:, cl, :],
                            rhs=gt[:, cl, :],
                            start=(k == 0),
                            stop=(k == total_k - 1),
                        )
                        k += 1
                rows = min(P, EPC - w * P)
                st = sep.tile([P, P], bf16, tag="seflush")
                nc.vector.tensor_copy(out=st[:], in_=psum_w[:])
                fl = nc.sync.dma_start(out=se_slice[w * P:w * P + rows, :], in_=st[:rows, :])
                if w * P < H:
                    flushes_h1.append(fl)
                if w * P + rows > H:
                    flushes_h2.append(fl)
                if w == w_ag1:
                    ag1 = nc.gpsimd.collective_compute(
                        "AllGather", mybir.AluOpType.bypass,
                        replica_groups=[list(range(ncores))],
                        ins=[se_slice[0:H, :]], outs=[se_h1[:]])
                    for f in flushes_h1:
                        add_dep_helper(ag1.ins, f.ins, reason="AG1 reads se_slice[:H]")
            ag2 = nc.gpsimd.collective_compute(
                "AllGather", mybir.AluOpType.bypass,
                replica_groups=[list(range(ncores))],
                ins=[se_slice[H:EPC, :]], outs=[se_h2[:]])
            for f in flushes_h2:
                add_dep_helper(ag2.ins, f.ins, reason="AG2 reads se_slice[H:]")

            # ---- stage B: T[v] = sum_{pairs with vertex v} Se[e] (transposed)
            # pass A over half-1 edges (after AG1), pass B adds half-2 (after AG2),
            # with the dense tail (stages C/D) interleaved as windows finalize.
            s3a = Stream("s3a", s3a_idx, s3a_seg, S3A, se_h1[:], C3a)
            s3b = Stream("s3b", s3b_idx, s3b_seg, S3B, se_h2[:], C3b)
            Tt = resident.tile([P, NW3 * P], f32, tag="Tt")
            xd_t = resident.tile([D, VPC], f32, tag="xd")
            nc.sync.dma_start(xd_t[:], xd_d[:])
            x0h_t = resident.tile([D, VPC], f32, tag="x0h")
            nc.sync.dma_start(x0h_t[:], x0h_d[:])
            zt_t = resident.tile([D, VPC], bf16, tag="zt")

            for w in range(NW3):
                total_k = int(s3a.counts[w])
                psum_w = winp.tile([P, P], f32, tag="win")
                for k, c in enumerate(range(int(s3a.off[w]), int(s3a.off[w]) + total_k)):
                    b, cl = divmod(c, G)
                    gt, oh = s3a.batch(b)
                    nc.tensor.matmul(
                        psum_w[:],
                        lhsT=gt[:, cl, :],
                        rhs=oh[:, cl, :],
                        start=(k == 0),
                        stop=(k == total_k - 1),
                    )
                nc.vector.tensor_copy(out=Tt[:, w * P:(w + 1) * P], in_=psum_w[:])

            RT = 512

            def emit_c_tile(rt):
                s0 = rt * RT
                L = min(RT, VPC - s0)
                pz = zvp.tile([P, RT], f32, tag="zv")
                nc.tensor.matmul(pz[:, :L], lhsT=wa_t[:], rhs=xd_t[:, s0:s0 + L],
                                 start=True, stop=False)
                nc.tensor.matmul(pz[:, :L], lhsT=wt_t[:], rhs=Tt[:, s0:s0 + L],
                                 start=False, stop=True)
                nc.vector.tensor_add(out=zt_t[:, s0:s0 + L], in0=pz[:, :L],
                                     in1=x0h_t[:, s0:s0 + L])
                for ot in range(s0 // P, (s0 + L + P - 1) // P):
                    o0 = ot * P
                    Lo = min(P, VPC - o0)
                    po = outp.tile([P, P], f32, tag="out")
                    nc.tensor.matmul(po[:Lo, :], lhsT=zt_t[:, o0:o0 + Lo], rhs=w3_t[:],
                                     start=True, stop=True)
                    st = sep.tile([P, P], f32, tag="outflush")
                    nc.vector.tensor_tensor(out=st[:Lo, :], in0=po[:Lo, :],
                                            in1=b3f_t[:Lo, :], op=mybir.AluOpType.add)
                    nc.sync.dma_start(out=out_d[o0:o0 + Lo, :], in_=st[:Lo, :])

            n_ctiles = (VPC + RT - 1) // RT
            done_c = 0
            for w in range(NW3):
                total_k = int(s3b.counts[w])
                if total_k > 0:
                    psum_w = winp.tile([P, P], f32, tag="win")
                    for k, c in enumerate(range(int(s3b.off[w]), int(s3b.off[w]) + total_k)):
                        b, cl = divmod(c, G)
                        gt, oh = s3b.batch(b)
                        nc.tensor.matmul(
                            psum_w[:],
                            lhsT=gt[:, cl, :],
                            rhs=oh[:, cl, :],
                            start=(k == 0),
                            stop=(k == total_k - 1),
                        )
                    nc.vector.tensor_add(out=Tt[:, w * P:(w + 1) * P],
                                         in0=Tt[:, w * P:(w + 1) * P], in1=psum_w[:])
                # emit any C tiles fully covered by finalized windows
                while done_c < n_ctiles and (done_c + 1) * RT <= (w + 1) * P:
                    emit_c_tile(done_c)
                    done_c += 1
            while done_c < n_ctiles:
                emit_c_tile(done_c)
                done_c += 1

            for inst in s3a.gather_insts:
                add_dep_helper(inst.ins, ag1.ins, reason="pass-A gathers read se_h1")
            for inst in s3b.gather_insts:
                add_dep_helper(inst.ins, ag2.ins, reason="pass-B gathers read se_h2")

    return nc


# ---------------------------------------------------------------------------
# entry point
# ---------------------------------------------------------------------------

def _run(inputs, trace=False, M=25000, ncores=8, G=8, lo_split=32768):
    import sys
    if "/opt/trn_rl_repo" not in sys.path:
        sys.path.insert(0, "/opt/trn_rl_repo")
    from concourse.bass_utils import run_bass_kernel_spmd

    in_maps, meta = preprocess(**inputs, M=M, ncores=ncores, G=G, lo_split=lo_split)
    nc = build_program(meta)
    if not nc.is_finalized():
        nc.finalize()
    res = run_bass_kernel_spmd(nc, in_maps, list(range(ncores)), trace=trace)
    out = np.concatenate([np.asarray(res.results[i]["out"]) for i in range(ncores)], axis=0)
    return out, res


def kernel(**inputs):
    out, _ = _run(inputs)
    return out


# revision 17
# speedup vs baseline: 3.6253x; 1.1147x over previous
"""Trainium2 Bass kernel for nn_EquivSetConv (hypergraph message passing).

Reference computation:
    Xve = (X @ W1 + b1)[vertex]
    Xe  = segment_sum(Xve, edges, M)
    Xev = Xe[edges]
    H   = concat([X[vertex], Xev], -1) @ W2 + b2
    Xv  = segment_sum(H, vertex, N)
    out = ((1-a)*Xv + a*X0) @ W3 + b3

Algebraic restructure (A[v,e] = #incidence pairs (v,e)):
    Se  = A^T @ X                          (segmented sum of raw X rows per edge)
    Xe  = Se @ W1 + edeg x b1
    T   = A @ Se                           (segmented sum of Se rows per vertex)
    Xv  = deg . (X @ W2a) + T @ (W1 @ W2b) + deg x b2 + wdeg x (b1 @ W2b)
    out = ((1-a)Xv + a X0) @ W3 + b3

So the 800k-row dense matmul disappears; the kernel is two sparse
gather+segmented-sum stages plus small dense matmuls.

Sharding over 8 cores: stage A partitioned by edge range (each core owns
M/8 edges and all pairs incident to them -> computes its Se slice fully,
no cross-core reduction), one AllGather of Se, stage B partitioned by
vertex range (each core owns N/8 vertices -> computes its output rows
end to end). The only collective is the 1.6MB/rank AllGather.

Sparse stages on device: host sorts pairs by destination segment and
packs them into 128-pair chunks that are pure in a 128-wide segment
window.  For each chunk: dma_gather 128 source rows (pair p -> SBUF
partition p), build a one-hot [pair, segment-slot] matrix on DVE
(iota == seg), and accumulate with one PE matmul into the window's PSUM
tile.  Windows flush to SBUF/DRAM when complete.
"""

import numpy as np

P = 128
D = 128


# ---------------------------------------------------------------------------
# host-side preprocessing
# ---------------------------------------------------------------------------

def _pack_stream(seg_local, gidx, n_windows, chunk_counts):
    """Pack pairs (sorted by window) into window-pure 128-slot chunks.

    seg_local: [n] int, segment id LOCAL to the stream's window grid
               (seg_local // 128 = window, seg_local % 128 = slot)
    gidx:      [n] int, gather index of each pair
    chunk_counts: [n_windows] int, chunks allocated per window (shared
               across all cores so the program structure is identical).

    Returns (idx16, segf) flat arrays of length sum(chunk_counts)*128,
    pad slots have idx 0 / seg -1.
    """
    total_chunks = int(np.sum(chunk_counts))
    tot = total_chunks * P
    idx16 = np.zeros(tot, dtype=np.int16)
    segf = np.full(tot, -1.0, dtype=np.float32)
    if len(seg_local) == 0:
        return idx16, segf

    order = np.argsort(seg_local, kind="stable")
    seg_s = seg_local[order]
    gidx_s = gidx[order]
    win = seg_s // P

    # position of each pair: chunk_base[win]*128 + rank-within-window
    chunk_base = np.concatenate([[0], np.cumsum(chunk_counts)[:-1]])
    win_start = np.searchsorted(win, np.arange(n_windows), side="left")
    rank = np.arange(len(win)) - win_start[win]
    pos = chunk_base[win] * P + rank
    idx16[pos] = gidx_s.astype(np.int16)
    segf[pos] = (seg_s % P).astype(np.float32)
    return idx16, segf


def _wrap_idx(idx16, G):
    """Reshape a flat per-stream idx array into the dma_gather SBUF layout.

    Within each batch of G*128 indices, index i lives at
    [partition i%16, column i//16]; batches are side by side.
    Output [128, total_chunks*8] int16 (rows 0..15 replicated to 128).
    """
    nb = len(idx16) // (G * P)
    blocks = [idx16[b * G * P:(b + 1) * G * P].reshape(G * 8, 16).T for b in range(nb)]
    arr16 = np.hstack(blocks)  # [16, total_chunks*8]
    return np.tile(arr16, (8, 1)).astype(np.int16)


def _seg_tile(segf):
    """[total_chunks*128] -> [128, total_chunks]: pair (chunk c, part p)."""
    import ml_dtypes
    n_chunks = len(segf) // P
    return np.ascontiguousarray(segf.reshape(n_chunks, P).T).astype(ml_dtypes.bfloat16)


def _chunk_counts(windows_per_core, n_windows, G, min_one=True):
    """windows_per_core: list over cores of [n_windows] pair counts.
    Returns per-window chunk counts (max over cores), padded so the
    total is a multiple of G."""
    counts = np.zeros(n_windows, dtype=np.int64)
    for wc in windows_per_core:
        counts = np.maximum(counts, (wc + P - 1) // P)
    if min_one:
        counts = np.maximum(counts, 1)
    tot = int(counts.sum())
    rem = (-tot) % G
    counts[-1] += rem
    return counts


def preprocess(X, vertex, edges, X0, W1, b1, W2, b2, W3, b3,
               M=25000, ncores=8, G=8, lo_split=32768):
    """Build per-core input maps + compile-time metadata."""
    X = np.asarray(X, dtype=np.float32)
    X0 = np.asarray(X0, dtype=np.float32)
    vertex = np.asarray(vertex).astype(np.int64)
    edges = np.asarray(edges).astype(np.int64)
    W1 = np.asarray(W1, dtype=np.float32)
    b1 = np.asarray(b1, dtype=np.float32)
    W2 = np.asarray(W2, dtype=np.float32)
    b2 = np.asarray(b2, dtype=np.float32)
    W3 = np.asarray(W3, dtype=np.float32)
    b3 = np.asarray(b3, dtype=np.float32)

    N, Din = X.shape
    NNZ = len(vertex)
    Dout = W3.shape[1]
    assert Din == D and Dout == D

    LO = min(lo_split, N)
    EPC = M // ncores
    VPC = N // ncores
    assert M % ncores == 0 and N % ncores == 0
    NW2 = (EPC + P - 1) // P
    NW3 = (VPC + P - 1) // P

    alpha = 0.5
    W2a = W2[:D]
    W2b = W2[D:]
    deg = np.bincount(vertex, minlength=N).astype(np.float64)
    edeg = np.bincount(edges, minlength=M).astype(np.float64)
    wdeg = np.bincount(vertex, weights=edeg[edges], minlength=N)

    Wa = ((1.0 - alpha) * W2a).astype(np.float32)
    Wt = ((1.0 - alpha) * (W1.astype(np.float64) @ W2b.astype(np.float64))).astype(np.float32)
    b1w = (W2b.astype(np.float64).T @ b1.astype(np.float64))  # b1 @ W2b
    b3_full = np.tile(b3[None, :], (P, 1)).astype(np.float32)

    core_edge = edges // EPC
    core_vert = vertex // VPC

    H = (EPC // 2 // P) * P if EPC >= 2 * P else max(EPC // 2, 1)

    # ---- per-core pair lists
    s2lo_w, s2hi_w, s3a_w, s3b_w = [], [], [], []
    s2lo_pairs, s2hi_pairs, s3a_pairs, s3b_pairs = [], [], [], []
    for i in range(ncores):
        sel = np.nonzero(core_edge == i)[0]
        pv = vertex[sel]
        pe = edges[sel] - i * EPC
        mlo = pv < LO
        for store_w, store_p, v, e in (
            (s2lo_w, s2lo_pairs, pv[mlo], pe[mlo]),
            (s2hi_w, s2hi_pairs, pv[~mlo] - LO, pe[~mlo]),
        ):
            store_w.append(np.bincount(e // P, minlength=NW2))
            store_p.append((e, v))
        sel = np.nonzero(core_vert == i)[0]
        pe = edges[sel]
        pvl = vertex[sel] - i * VPC
        el = pe % EPC
        er = pe // EPC
        mha = el < H
        # remapped gather indices into the two half-AllGather layouts
        ia = er[mha] * H + el[mha]
        ib = er[~mha] * (EPC - H) + (el[~mha] - H)
        s3a_w.append(np.bincount(pvl[mha] // P, minlength=NW3))
        s3b_w.append(np.bincount(pvl[~mha] // P, minlength=NW3))
        s3a_pairs.append((pvl[mha], ia))
        s3b_pairs.append((pvl[~mha], ib))

    C2lo = _chunk_counts(s2lo_w, NW2, G)
    C2hi = _chunk_counts(s2hi_w, NW2, G)
    C3a = _chunk_counts(s3a_w, NW3, G)
    C3b = _chunk_counts(s3b_w, NW3, G, min_one=False)
    S2LO, S2HI = int(C2lo.sum()), int(C2hi.sum())
    S3A, S3B = int(C3a.sum()), int(C3b.sum())

    iota = np.tile(np.arange(P, dtype=np.float32), G)[None, :].repeat(P, axis=0)
    iota = np.ascontiguousarray(iota)

    import ml_dtypes
    X_bf16 = X.astype(ml_dtypes.bfloat16)
    iota = iota.astype(ml_dtypes.bfloat16)
    W3h = W3.astype(ml_dtypes.bfloat16)

    in_maps = []
    for i in range(ncores):
        e, v = s2lo_pairs[i]
        lo_idx, lo_seg = _pack_stream(e, v, NW2, C2lo)
        e, v = s2hi_pairs[i]
        hi_idx, hi_seg = _pack_stream(e, v, NW2, C2hi)
        pvl, ia = s3a_pairs[i]
        s3a_idx, s3a_seg = _pack_stream(pvl, ia, NW3, C3a)
        pvl, ib = s3b_pairs[i]
        s3b_idx, s3b_seg = _pack_stream(pvl, ib, NW3, C3b)

        sl = slice(i * VPC, (i + 1) * VPC)
        xd_t = np.ascontiguousarray((X[sl].astype(np.float64) * deg[sl, None]).T).astype(np.float32)
        x0h = alpha * X0[sl].astype(np.float64).T \
            + (1.0 - alpha) * (np.outer(b2, deg[sl]) + np.outer(b1w, wdeg[sl]))
        x0h_t = np.ascontiguousarray(x0h).astype(np.float32)

        in_maps.append({
            "x_tab": X_bf16,
            "s2lo_idx": _wrap_idx(lo_idx, G), "s2lo_seg": _seg_tile(lo_seg),
            "s2hi_idx": _wrap_idx(hi_idx, G), "s2hi_seg": _seg_tile(hi_seg),
            "s3a_idx": _wrap_idx(s3a_idx, G), "s3a_seg": _seg_tile(s3a_seg),
            "s3b_idx": _wrap_idx(s3b_idx, G), "s3b_seg": _seg_tile(s3b_seg),
            "iota": iota,
            "xd_t": xd_t,
            "x0h_t": x0h_t,
            "wa": Wa, "wt": Wt, "w3": W3h, "b3f": b3_full,
        })

    meta = dict(N=N, M=M, NNZ=NNZ, ncores=ncores, G=G, LO=LO, H=H,
                EPC=EPC, VPC=VPC, NW2=NW2, NW3=NW3,
                C2lo=C2lo.tolist(), C2hi=C2hi.tolist(),
                C3a=C3a.tolist(), C3b=C3b.tolist(),
                S2LO=S2LO, S2HI=S2HI, S3A=S3A, S3B=S3B)
    return in_maps, meta


# ---------------------------------------------------------------------------
# device program
# ---------------------------------------------------------------------------

def build_program(meta):
    import concourse.bacc as bacc
    import concourse.bass as bass  # noqa: F401
    import concourse.mybir as mybir
    import concourse.tile as tile
    from concourse._compat import get_trn_type
    from concourse import library_config
    from concourse.tile_rust import add_dep_helper

    f32 = mybir.dt.float32
    bf16 = mybir.dt.bfloat16
    i16 = mybir.dt.int16

    ncores = meta["ncores"]
    G = meta["G"]
    N, M = meta["N"], meta["M"]
    LO = meta["LO"]
    EPC, VPC = meta["EPC"], meta["VPC"]
    NW2, NW3 = meta["NW2"], meta["NW3"]
    C2lo, C2hi = meta["C2lo"], meta["C2hi"]
    C3a, C3b = meta["C3a"], meta["C3b"]
    S2LO, S2HI = meta["S2LO"], meta["S2HI"]
    S3A, S3B = meta["S3A"], meta["S3B"]
    H = meta["H"]
    GP = G * P

    nc = bacc.Bacc(get_trn_type() or "TRN2", num_devices=ncores, num_swdge_queues=4)

    x_tab = nc.declare_dram_parameter("x_tab", [N, D], bf16, isOutput=False)
    s2lo_idx = nc.declare_dram_parameter("s2lo_idx", [P, S2LO * 8], i16, isOutput=False)
    s2lo_seg = nc.declare_dram_parameter("s2lo_seg", [P, S2LO], bf16, isOutput=False)
    s2hi_idx = nc.declare_dram_parameter("s2hi_idx", [P, S2HI * 8], i16, isOutput=False)
    s2hi_seg = nc.declare_dram_parameter("s2hi_seg", [P, S2HI], bf16, isOutput=False)
    s3a_idx = nc.declare_dram_parameter("s3a_idx", [P, S3A * 8], i16, isOutput=False)
    s3a_seg = nc.declare_dram_parameter("s3a_seg", [P, S3A], bf16, isOutput=False)
    s3b_idx = nc.declare_dram_parameter("s3b_idx", [P, S3B * 8], i16, isOutput=False)
    s3b_seg = nc.declare_dram_parameter("s3b_seg", [P, S3B], bf16, isOutput=False)
    iota_d = nc.declare_dram_parameter("iota", [P, GP], bf16, isOutput=False)
    xd_d = nc.declare_dram_parameter("xd_t", [D, VPC], f32, isOutput=False)
    x0h_d = nc.declare_dram_parameter("x0h_t", [D, VPC], f32, isOutput=False)
    wa_d = nc.declare_dram_parameter("wa", [D, D], f32, isOutput=False)
    wt_d = nc.declare_dram_parameter("wt", [D, D], f32, isOutput=False)
    w3_d = nc.declare_dram_parameter("w3", [D, D], bf16, isOutput=False)
    b3f_d = nc.declare_dram_parameter("b3f", [P, D], f32, isOutput=False)
    out_d = nc.declare_dram_parameter("out", [VPC, D], f32, isOutput=True)

    se_slice = nc.dram_tensor("se_slice", [EPC, D], bf16)
    se_h1 = nc.dram_tensor("se_h1", [ncores * H, D], bf16, addr_space="Shared")
    se_h2 = nc.dram_tensor("se_h2", [ncores * (EPC - H), D], bf16, addr_space="Shared")

    with tile.TileContext(nc) as tc:
        with (
            tc.tile_pool(name="consts", bufs=1) as consts,
            tc.tile_pool(name="resident", bufs=1) as resident,
            tc.tile_pool(name="gat", bufs=8) as gat,
            tc.tile_pool(name="ohp", bufs=8) as ohp,
            tc.tile_pool(name="sep", bufs=3) as sep,
            tc.tile_pool(name="winp", bufs=5, space="PSUM") as winp,
            tc.tile_pool(name="zvp", bufs=1, space="PSUM") as zvp,
            tc.tile_pool(name="outp", bufs=2, space="PSUM") as outp,
        ):
            # ---- resident loads
            iota_t = consts.tile([P, G, P], bf16)
            nc.sync.dma_start(iota_t[:], iota_d[:].rearrange("p (g q) -> p g q", q=P))
            wa_t = consts.tile([D, D], f32)
            nc.sync.dma_start(wa_t[:], wa_d[:])
            wt_t = consts.tile([D, D], f32)
            nc.sync.dma_start(wt_t[:], wt_d[:])
            w3_t = consts.tile([D, D], bf16)
            nc.sync.dma_start(w3_t[:], w3_d[:])
            b3f_t = consts.tile([P, D], f32)
            nc.sync.dma_start(b3f_t[:], b3f_d[:])

            nc.gpsimd.load_library(library_config.mlp)
            npairs_reg = nc.gpsimd.to_reg(GP)
            qctr = [0]

            class Stream:
                def __init__(self, name, idx_d, seg_d, n_chunks, table_ap, counts):
                    self.name = name
                    self.counts = counts
                    self.off = np.concatenate([[0], np.cumsum(counts)[:-1]]).astype(int)
                    self.table_ap = table_ap
                    self.idx_t = resident.tile([P, n_chunks * 8], i16, tag=f"idx_{name}")
                    nc.sync.dma_start(self.idx_t[:], idx_d[:])
                    self.seg_t = resident.tile([P, n_chunks], bf16, tag=f"seg_{name}")
                    nc.sync.dma_start(self.seg_t[:], seg_d[:])
                    self.batches = {}
                    self.gather_insts = []

                def batch(self, b):
                    if b not in self.batches:
                        gt = gat.tile([P, G, D], bf16, tag="gat")
                        inst = nc.gpsimd.dma_gather(
                            gt[:],
                            self.table_ap,
                            self.idx_t[:, b * G * 8:(b + 1) * G * 8],
                            GP,
                            npairs_reg,
                            D,
                            queue_num=qctr[0] % 4,
                        )
                        qctr[0] += 1
                        self.gather_insts.append(inst)
                        oh = ohp.tile([P, G, P], bf16, tag="oh")
                        nc.vector.tensor_tensor(
                            out=oh[:],
                            in0=iota_t[:],
                            in1=self.seg_t[:, b * G:(b + 1) * G].broadcast_to([P, G, P]),
                            op=mybir.AluOpType.is_equal,
                        )
                        self.batches[b] = (gt, oh)
                    return self.batches[b]

            lo = Stream("s2lo", s2lo_idx, s2lo_seg, S2LO, x_tab[0:LO, :], C2lo)
            streams2 = [lo]
            if LO < N:
                hi = Stream("s2hi", s2hi_idx, s2hi_seg, S2HI, x_tab[LO:N, :], C2hi)
                streams2.append(hi)

            # ---- stage A: Se[e] = sum_{pairs with edge e} X[v]
            flushes_h1 = []
            flushes_h2 = []
            ag1 = ag2 = None
            w_ag1 = (H - 1) // P  # AG1 after this window's flush
            for w in range(NW2):
                total_k = sum(int(s.counts[w]) for s in streams2)
                psum_w = winp.tile([P, P], f32, tag="win")
                k = 0
                for s in streams2:
                    for c in range(int(s.off[w]), int(s.off[w]) + int(s.counts[w])):
                        b, cl = divmod(c, G)
                        gt, oh = s.batch(b)
                        nc.tensor.matmul(
                            psum_w[:],
                            lhsT=oh[:, cl, :],
                            rhs=gt[:, cl, :],
                            start=(k == 0),
                            stop=(k == total_k - 1),
                        )
                        k += 1
                rows = min(P, EPC - w * P)
                st = sep.tile([P, P], bf16, tag="seflush")
                nc.vector.tensor_copy(out=st[:], in_=psum_w[:])
                fl = nc.sync.dma_start(out=se_slice[w * P:w * P + rows, :], in_=st[:rows, :])
                if w * P < H:
                    flushes_h1.append(fl)
                if w * P + rows > H:
                    flushes_h2.append(fl)
                if w == w_ag1:
                    ag1 = nc.gpsimd.collective_compute(
                        "AllGather", mybir.AluOpType.bypass,
                        replica_groups=[list(range(ncores))],
                        ins=[se_slice[0:H, :]], outs=[se_h1[:]])
                    for f in flushes_h1:
                        add_dep_helper(ag1.ins, f.ins, reason="AG1 reads se_slice[:H]")
            ag2 = nc.gpsimd.collective_compute(
                "AllGather", mybir.AluOpType.bypass,
                replica_groups=[list(range(ncores))],
                ins=[se_slice[H:EPC, :]], outs=[se_h2[:]])
            for f in flushes_h2:
                add_dep_helper(ag2.ins, f.ins, reason="AG2 reads se_slice[H:]")

            # ---- stage B: T[v] = sum_{pairs with vertex v} Se[e] (transposed)
            # pass A over half-1 edges (after AG1), pass B adds half-2 (after AG2),
            # with the dense tail (stages C/D) interleaved as windows finalize.
            s3a = Stream("s3a", s3a_idx, s3a_seg, S3A, se_h1[:], C3a)
            s3b = Stream("s3b", s3b_idx, s3b_seg, S3B, se_h2[:], C3b)
            Tt = resident.tile([P, NW3 * P], f32, tag="Tt")
            xd_t = resident.tile([D, VPC], f32, tag="xd")
            nc.sync.dma_start(xd_t[:], xd_d[:])
            x0h_t = resident.tile([D, VPC], f32, tag="x0h")
            nc.sync.dma_start(x0h_t[:], x0h_d[:])
            zt_t = resident.tile([D, VPC], bf16, tag="zt")

            for w in range(NW3):
                total_k = int(s3a.counts[w])
                psum_w = winp.tile([P, P], f32, tag="win")
                for k, c in enumerate(range(int(s3a.off[w]), int(s3a.off[w]) + total_k)):
                    b, cl = divmod(c, G)
                    gt, oh = s3a.batch(b)
                    nc.tensor.matmul(
                        psum_w[:],
                        lhsT=gt[:, cl, :],
                        rhs=oh[:, cl, :],
                        start=(k == 0),
                        stop=(k == total_k - 1),
                    )
                nc.vector.tensor_copy(out=Tt[:, w * P:(w + 1) * P], in_=psum_w[:])

            RT = 512

            def emit_c_tile(rt):
                s0 = rt * RT
                L = min(RT, VPC - s0)
                pz = zvp.tile([P, RT], f32, tag="zv")
                nc.tensor.matmul(pz[:, :L], lhsT=wa_t[:], rhs=xd_t[:, s0:s0 + L],
                                 start=True, stop=False)
                nc.tensor.matmul(pz[:, :L], lhsT=wt_t[:], rhs=Tt[:, s0:s0 + L],
                                 start=False, stop=True)
                nc.vector.tensor_add(out=zt_t[:, s0:s0 + L], in0=pz[:, :L],
                                     in1=x0h_t[:, s0:s0 + L])
                for ot in range(s0 // P, (s0 + L + P - 1) // P):
                    o0 = ot * P
                    Lo = min(P, VPC - o0)
                    po = outp.tile([P, P], f32, tag="out")
                    nc.tensor.matmul(po[:Lo, :], lhsT=zt_t[:, o0:o0 + Lo], rhs=w3_t[:],
                                     start=True, stop=True)
                    st = sep.tile([P, P], f32, tag="outflush")
                    nc.vector.tensor_tensor(out=st[:Lo, :], in0=po[:Lo, :],
                                            in1=b3f_t[:Lo, :], op=mybir.AluOpType.add)
                    nc.sync.dma_start(out=out_d[o0:o0 + Lo, :], in_=st[:Lo, :])

            n_ctiles = (VPC + RT - 1) // RT
            done_c = 0
            for w in range(NW3):
                total_k = int(s3b.counts[w])
                if total_k > 0:
                    psum_w = winp.tile([P, P], f32, tag="win")
                    for k, c in enumerate(range(int(s3b.off[w]), int(s3b.off[w]) + total_k)):
                        b, cl = divmod(c, G)
                        gt, oh = s3b.batch(b)
                        nc.tensor.matmul(
                            psum_w[:],
                            lhsT=gt[:, cl, :],
                            rhs=oh[:, cl, :],
                            start=(k == 0),
                            stop=(k == total_k - 1),
                        )
                    nc.vector.tensor_add(out=Tt[:, w * P:(w + 1) * P],
                                         in0=Tt[:, w * P:(w + 1) * P], in1=psum_w[:])
                # emit any C tiles fully covered by finalized windows
                while done_c < n_ctiles and (done_c + 1) * RT <= (w + 1) * P:
                    emit_c_tile(done_c)
                    done_c += 1
            while done_c < n_ctiles:
                emit_c_tile(done_c)
                done_c += 1

            for inst in s3a.gather_insts:
                add_dep_helper(inst.ins, ag1.ins, reason="pass-A gathers read se_h1")
            for inst in s3b.gather_insts:
                add_dep_helper(inst.ins, ag2.ins, reason="pass-B gathers read se_h2")

    return nc


# ---------------------------------------------------------------------------
# entry point
# ---------------------------------------------------------------------------

def _run(inputs, trace=False, M=25000, ncores=8, G=8, lo_split=32768):
    import sys
    if "/opt/trn_rl_repo" not in sys.path:
        sys.path.insert(0, "/opt/trn_rl_repo")
    from concourse.bass_utils import run_bass_kernel_spmd

    in_maps, meta = preprocess(**inputs, M=M, ncores=ncores, G=G, lo_split=lo_split)
    nc = build_program(meta)
    if not nc.is_finalized():
        nc.finalize()
    res = run_bass_kernel_spmd(nc, in_maps, list(range(ncores)), trace=trace)
    out = np.concatenate([np.asarray(res.results[i]["out"]) for i in range(ncores)], axis=0)
    return out, res


def kernel(**inputs):
    out, _ = _run(inputs)
    return out
